# revision 39
# baseline (speedup 1.0000x reference)
"""Trainium2 Bass kernel for multi-head self-attention with RoPE.

Sharding: 8 cores = 2 (batch) x 4 (head groups of 4 heads).
Each core computes its batch's attention for its 4 heads plus the
(row-sharded) output projection partial sum; the host adds the 4 head-group
partials per batch and the output bias.

Schedule (single Tile program, engines self-synchronize):
  S0:      head-0 q/k projection t-outer across 8 PSUM banks so the PE
           consumes hidden-state tiles as the DMAs land; then v projection.
  stage j: head j's q/k projection matmuls are interleaved between head
           j-1's score matmuls as PE filler while the scalar engine
           computes exp() of the score tiles; per (chunk, head) the softmax
           denominator is a vector-engine tree-sum of the exp tiles plus a
           single ones-matmul broadcast, inverted with vector reciprocal
           (no Ln -> single ACT table, no table thrash).
  final:   head-3 attention chunk-by-chunk with the previous chunk's
           output-projection matmuls as PE filler.

The causal mask multiply uses one shared [128, 4*512] diagonal block
(identical for every query chunk) applied as a single tensor_mul per
(chunk, head). Output is stored fp16; host adds partials in fp32.
"""

import sys
import types
from collections import deque

import numpy as np

sys.path.insert(0, "/opt/trn_rl_repo")

# The axon boot registers its NTFF-profiling hook via antenv.axon_hooks; some
# images lack that module, which silently disables tracing. Provide it.
if "antenv.axon_hooks" not in sys.modules:
    try:
        import antenv.axon_hooks  # noqa: F401
    except ImportError:
        try:
            import antenv

            _m = types.ModuleType("antenv.axon_hooks")
            _m._hook = None
            _m.set_axon_ntff_profile_hook = lambda h: setattr(_m, "_hook", h)
            _m.get_axon_ntff_profile_hook = lambda: _m._hook
            sys.modules["antenv.axon_hooks"] = _m
            antenv.axon_hooks = _m
        except ImportError:
            pass

B, S, H, NH, HD = 2, 2048, 2048, 16, 128
ROPE_THETA = 10000.0
N_CORES = 8
HGRID = 4            # head-group shards
NHC = NH // HGRID    # heads per core

LAST_RESULTS = None  # test harness introspection
_CACHE = {}


def _rope_tables(S_, dtype=np.float16):
    # transposed rope tables [HD, S]; ss has rotate-half sign folded in:
    # rope(x)[d, s] = x[d, s]*cosT[d, s] + x[(d+64)%128, s]*ss[d, s]
    inv = 1.0 / (ROPE_THETA ** (np.arange(0, HD, 2, dtype=np.float64) / HD))
    t = np.arange(S_, dtype=np.float64)
    fr = np.outer(t, inv)                          # [S, HD/2]
    emb = np.concatenate([fr, fr], axis=1)         # [S, HD]
    cosT = np.cos(emb).T.astype(np.float32)        # [HD, S]
    ss = np.sin(emb).T.astype(np.float32)
    ss[: HD // 2] *= -1.0
    return cosT.astype(dtype), ss.astype(dtype)


def build_program(S_, H_, NHC_, zero_bias=False):
    """Build + compile the per-core SPMD bass program (causal schedule)."""
    from contextlib import ExitStack

    import concourse.mybir as mybir
    import concourse.tile as tile
    from concourse import bacc

    f16 = mybir.dt.float16
    f32 = mybir.dt.float32
    AF = mybir.ActivationFunctionType

    T = H_ // 128       # hidden contraction tiles
    KT = S_ // 128      # key/seq tiles
    CC = S_ // 512      # query chunks
    HC = H_ // 512      # output hidden chunks
    SCALE = 1.0 / float(np.sqrt(HD))

    nc = bacc.Bacc("TRN2", target_bir_lowering=False, debug=False)

    hT_d = nc.dram_tensor("hT", [T, 128, S_], f16, kind="ExternalInput").ap()
    wq_d = nc.dram_tensor("wq", [NHC_, 128, T * HD], f16, kind="ExternalInput").ap()
    wk_d = nc.dram_tensor("wk", [NHC_, 128, T * HD], f16, kind="ExternalInput").ap()
    # wv/wo are pre-transposed host-side into SBUF layout (contiguous DMA)
    wv_d = nc.dram_tensor("wv", [128, T * NHC_ * HD], f16, kind="ExternalInput").ap()
    wo_d = nc.dram_tensor("wo", [128, NHC_ * H_], f16, kind="ExternalInput").ap()
    cos_d = nc.dram_tensor("cosT", [128, S_], f16, kind="ExternalInput").ap()
    ss_d = nc.dram_tensor("ssT", [128, S_], f16, kind="ExternalInput").ap()
    bq_d = nc.dram_tensor("bqT", [128, NHC_], f32, kind="ExternalInput").ap()
    bk_d = nc.dram_tensor("bkT", [128, NHC_], f32, kind="ExternalInput").ap()
    bv_d = nc.dram_tensor("bv4", [1, NHC_ * HD], f16, kind="ExternalInput").ap()
    md_d = nc.dram_tensor("mdiag", [128, 4 * 512], f16, kind="ExternalInput").ap()
    o_d = nc.dram_tensor("o", [S_, H_], f16, kind="ExternalOutput").ap()

    with ExitStack() as ctx:
        tc = ctx.enter_context(tile.TileContext(nc))
        persist = ctx.enter_context(tc.tile_pool(name="persist", bufs=1))

        # qr/kr 2-deep rings: stage j writes ring[j % 2]; head j's attention
        # reads it during stage j+1.
        qr = [persist.tile([128, S_], f16, name=f"qr{r}") for r in range(2)]
        kr = [persist.tile([128, S_], f16, name=f"kr{r}") for r in range(2)]
        vs = persist.tile([128, KT * NHC_ * HD], f16, name="vs")
        wo_sb = persist.tile([128, NHC_ * H_], f16, name="wo_sb")
        attnT = [persist.tile([128, NHC_ * 512], f16, name=f"attnT{c}")
                 for c in range(CC)]
        cos_sb = persist.tile([128, S_], f16, name="cos_sb")
        ss_sb = persist.tile([128, S_], f16, name="ss_sb")
        md_sb = persist.tile([128, 4 * 512], f16, name="md_sb")
        ones_sb = persist.tile([128, 128], f16, name="ones_sb")
        ones1 = persist.tile([1, 128], f16, name="ones1")
        bv_sb = persist.tile([1, NHC_ * HD], f16, name="bv_sb")
        bq_sb = persist.tile([128, NHC_], f32, name="bq_sb")
        bk_sb = persist.tile([128, NHC_], f32, name="bk_sb")
        # single staging buffer for pre-rope q/k (vector-queue order makes
        # reuse across stages safe: rope(j) reads are issued before stage
        # j+1's evac writes on the same queue)
        qs_st = persist.tile([128, S_], f16, name="qs_st")
        ks_st = persist.tile([128, S_], f16, name="ks_st")

        nc.vector.memset(ones_sb, 1.0)
        nc.vector.memset(ones1, 1.0)

        # per-head q/k weight tiles, 4-deep rotation (j and j+1 in flight)
        wpool = ctx.enter_context(tc.tile_pool(name="wpool", bufs=1))

        # ---- DMA issue (3 dynamic queues: sync, gpsimd, scalar) ----
        # Startup transfers are quartered (128KB) and dealt round-robin to
        # the three queues in PE-need order, so the first matmul's inputs
        # land within ~1us of queue start and the t-loop streams.
        hT_pool = ctx.enter_context(tc.tile_pool(name="hTp", bufs=1))
        # hT_sb[t][g]: quarter tiles [128, 512] (g = column group)
        hT_sb = [[hT_pool.tile([128, 512], f16, name=f"hT{t}_{g}")
                  for g in range(4)] for t in range(T)]
        # head-0 q/k weights quartered: w0x[g] holds t-slices 4g..4g+3
        # all q/k weights stored as 4 quarter-tiles [128, 512] per (head,
        # q/k), sharing one 16-deep rotation (two heads in flight)
        wst = {}

        def walloc(j, nm):
            wst[(j, nm)] = [wpool.tile([128, 512], f16, tag="w", bufs=16,
                                       name=f"w{j}{nm}_{g}") for g in range(4)]

        walloc(0, "q")
        walloc(0, "k")
        walloc(1, "q")
        walloc(1, "k")

        def wslice(j, nm, t):
            return wst[(j, nm)][t // 4][:, (t % 4) * HD:(t % 4 + 1) * HD]

        # small tensors ride early prio slots: the head-0 PSUM evacuations
        # need the biases, rope(0) needs cos/ss, well before the bulk
        # stream finishes
        prio = []   # (out_tile, dram_ap) in PE-need order
        for t in range(T):
            if t < 4:
                prio.append((wst[(0, "q")][t], wq_d[0][:, t * 512:(t + 1) * 512]))
                prio.append((wst[(0, "k")][t], wk_d[0][:, t * 512:(t + 1) * 512]))
            for g in range(4):
                prio.append((hT_sb[t][g], hT_d[t][:, g * 512:(g + 1) * 512]))
            if t == 0:
                prio.append((bq_sb, bq_d))
                prio.append((bk_sb, bk_d))
                prio.append((bv_sb, bv_d))
            if t == 5:      # rope tables, needed when the t-loop drains
                prio.append((cos_sb, cos_d))
                prio.append((ss_sb, ss_d))
            if t == 7:      # head-1 q weights land before the S0 hole-filler
                for g in range(4):
                    prio.append((wst[(1, "q")][g],
                                 wq_d[1][:, g * 512:(g + 1) * 512]))
            if t == 9:
                prio.append((md_sb, md_d))
            if t == 11:
                for g in range(4):
                    prio.append((wst[(1, "k")][g],
                                 wk_d[1][:, g * 512:(g + 1) * 512]))
        dma_eng = [nc.gpsimd, nc.scalar, nc.sync]
        for i, (out_t, in_ap) in enumerate(prio):
            dma_eng[i % 3].dma_start(out=out_t, in_=in_ap)

        ropep = ctx.enter_context(tc.tile_pool(name="ropep", bufs=1))
        wvp = ctx.enter_context(tc.tile_pool(name="wvp", bufs=1))
        wv_sb = wvp.tile([128, T * NHC_ * HD], f16, name="wv_sb")
        for g in range(4):
            sl = slice(g * T * NHC_ * HD // 4, (g + 1) * T * NHC_ * HD // 4)
            dma_eng[g % 3].dma_start(out=wv_sb[:, sl], in_=wv_d[:, sl])

        def load_w(j):
            for nm, w_d0 in (("q", wq_d), ("k", wk_d)):
                walloc(j, nm)
                for g in range(4):
                    nc.sync.dma_start(out=wst[(j, nm)][g],
                                      in_=w_d0[j][:, g * 512:(g + 1) * 512])

        for g in range(4):
            sl = slice(g * NHC_ * H_ // 4, (g + 1) * NHC_ * H_ // 4)
            nc.scalar.dma_start(out=wo_sb[:, sl], in_=wo_d[:, sl])

        # attnp/psm are created after the wv pool closes (SBUF/PSUM reuse);
        # attn_chunk binds them late, first use is after v-projection.

        def rope(j):
            """rope(qs_st/ks_st) -> qr/kr ring j%2 (vector + gpsimd shifts)."""
            r = j % 2
            for src, dst in ((qs_st, qr[r]), (ks_st, kr[r])):
                sh = ropep.tile([128, S_], f16, tag="sh", bufs=1, name="sh")
                acc = ropep.tile([128, S_], f16, tag="racc", bufs=1, name="racc")
                nc.gpsimd.dma_start(out=sh[0:64], in_=src[64:128])
                nc.gpsimd.dma_start(out=sh[64:128], in_=src[0:64])
                nc.vector.tensor_mul(acc, src, cos_sb)
                nc.vector.tensor_mul(dst, sh, ss_sb)
                nc.vector.tensor_add(dst, acc, dst)

        def proj_ops(j, parts=("q", "k"), pool=None):
            """PE-filler closures for head-j q/k projection (+ vector evac)."""
            for nm in parts:
                dstt = qs_st if nm == "q" else ks_st
                b_sb = bq_sb if nm == "q" else bk_sb
                for c in range(CC):
                    pp = pool.tile([128, 512], f32, tag="pp", bufs=2, name="pp")
                    for t in range(T):
                        yield lambda pp=pp, j=j, nm=nm, c=c, t=t: nc.tensor.matmul(
                            pp, lhsT=wslice(j, nm, t), rhs=hT_sb[t][c],
                            start=(t == 0), stop=(t == T - 1))
                    yield lambda pp=pp, dstt=dstt, c=c, b_sb=b_sb, j=j: \
                        nc.vector.tensor_scalar_add(
                            dstt[:, c * 512:(c + 1) * 512], pp, b_sb[:, j:j + 1])

        # ---------------- S0: head-0 q/k projection, t-outer ----------------
        with tc.tile_pool(name="ps0", bufs=1, space="PSUM") as ps0:
            ppq = [ps0.tile([128, 512], f32, tag="pj0", bufs=8, name=f"ppq{c}")
                   for c in range(CC)]
            ppk = [ps0.tile([128, 512], f32, tag="pj0", bufs=8, name=f"ppk{c}")
                   for c in range(CC)]
            for t in range(T):
                wq0 = wst[(0, "q")][t // 4][:, (t % 4) * HD:(t % 4 + 1) * HD]
                wk0 = wst[(0, "k")][t // 4][:, (t % 4) * HD:(t % 4 + 1) * HD]
                for c in range(CC):
                    nc.tensor.matmul(
                        ppq[c], lhsT=wq0, rhs=hT_sb[t][c],
                        start=(t == 0), stop=(t == T - 1))
                    nc.tensor.matmul(
                        ppk[c], lhsT=wk0, rhs=hT_sb[t][c],
                        start=(t == 0), stop=(t == T - 1))
            for c in range(CC):
                nc.vector.tensor_scalar_add(
                    qs_st[:, c * 512:(c + 1) * 512], ppq[c], bq_sb[:, 0:1])
                nc.vector.tensor_scalar_add(
                    ks_st[:, c * 512:(c + 1) * 512], ppk[c], bk_sb[:, 0:1])

        # rope(0) runs as soon as head-0 evacs land, freeing qs_st/ks_st
        # for the later heads' projections
        rope(0)

        psm = ctx.enter_context(tc.tile_pool(name="psm", bufs=1, space="PSUM"))

        def vproj_ops(sts):
            """v-projection ops for the given s-tiles (16 mm + bias + copy)."""
            for st in sts:
                vp = psm.tile([128, NHC_ * HD], f32, tag="pp", bufs=2,
                              name="vp")
                for t in range(T):
                    yield lambda vp=vp, st=st, t=t: nc.tensor.matmul(
                        vp,
                        lhsT=hT_sb[t][st // 4][:, (st % 4) * 128:
                                               (st % 4) * 128 + 128],
                        rhs=wv_sb[:, t * NHC_ * HD:(t + 1) * NHC_ * HD],
                        start=(t == 0),
                        stop=(zero_bias and t == T - 1))
                if not zero_bias:
                    yield lambda vp=vp: nc.tensor.matmul(
                        vp, lhsT=ones1, rhs=bv_sb, start=False, stop=True)
                yield lambda vp=vp, st=st: nc.vector.tensor_copy(
                    vs[:, st * NHC_ * HD:(st + 1) * NHC_ * HD], vp)

        # ---------------- S0 tail: head-1 projection + rope + v-proj front --
        # the input stream (hT+wv) is still landing; head-1's projection and
        # the front half of the v-projection keep the PE busy through it
        for f in proj_ops(1, ("q", "k"), pool=psm):
            f()
        rope(1)
        for f in vproj_ops(range(0, 4)):
            f()

        attnp = ctx.enter_context(tc.tile_pool(name="attnp", bufs=1))

        def attn_chunk(jj, c, pull, diag_first=True):
            """Head jj's chunk-c scores/exp/au/normalize; pull() issues
            PE-filler ops between score pairs. The au accumulation is
            interleaved per pair so each exp tile dies immediately
            (expT is only [128, 1024]); the softmax denominator is a
            running vector sum into expsum."""
            rr = jj % 2
            inc = 4 * (c + 1)           # causal: k-tiles 0..4c+3
            # diag_first: masked diagonal pairs first so the chunk tail has
            # no mask dependency; diag-last when the vector queue is busy
            # early in the chunk (rope overlap)
            if diag_first:
                order = [inc - 4, inc - 2] + list(range(0, inc - 4, 2))
            else:
                order = list(range(0, inc, 2))
            au = psm.tile([128, 512], f32, tag="au", bufs=2, name="au")
            expsum = attnp.tile([128, 512], f16, tag="expsum", bufs=2,
                                name="expsum")
            prev = None
            first_pair = order[0]

            def au_and_sum(q0, qexpT):
                for kk in (0, 1):
                    kt = q0 + kk
                    nc.tensor.matmul(
                        au,
                        lhsT=vs[:, (kt * NHC_ + jj) * HD:
                                (kt * NHC_ + jj + 1) * HD],
                        rhs=qexpT[:, kk * 512:(kk + 1) * 512],
                        start=(q0 == first_pair and kk == 0),
                        stop=(q0 == order[-1] and kk == 1))
                if q0 == first_pair:
                    nc.vector.tensor_add(
                        expsum, qexpT[:, 0:512], qexpT[:, 512:1024])
                else:
                    nc.vector.tensor_add(expsum, expsum, qexpT[:, 0:512])
                    nc.vector.tensor_add(expsum, expsum, qexpT[:, 512:1024])

            for p0 in order:
                scp = psm.tile([128, 1024], f32, tag="scp", bufs=2, name="scp")
                for kk in (0, 1):
                    kt = p0 + kk
                    nc.tensor.matmul(
                        scp[:, kk * 512:(kk + 1) * 512],
                        lhsT=kr[rr][:, kt * 128:(kt + 1) * 128],
                        rhs=qr[rr][:, c * 512:(c + 1) * 512],
                        start=True, stop=True)
                expT = attnp.tile([128, 1024], f16, tag="expT", bufs=3,
                                  name="expT")
                # fused scale+exp over the pair
                nc.scalar.activation(expT, scp, AF.Exp, scale=SCALE)
                if p0 >= inc - 4:
                    # multiplicative causal mask (diag tiles p0-(inc-4), +1)
                    i0 = (p0 - (inc - 4)) * 512
                    nc.vector.tensor_mul(expT, expT, md_sb[:, i0:i0 + 1024])
                # au accumulation for the previous pair (its exp is ready)
                if prev is not None:
                    au_and_sum(*prev)
                prev = (p0, expT)
                pull()
            au_and_sum(*prev)
            # denominator broadcast into a scp-tag buffer (au keeps 2-deep
            # rotation so the divide latency is off the PE critical path)
            rlb = psm.tile([128, 1024], f32, tag="scp", bufs=2, name="rlb")
            nc.tensor.matmul(rlb[:, 0:512], lhsT=ones_sb, rhs=expsum,
                             start=True, stop=True)
            # 1/l at ~18 bits via one custom-DVE op (plenty for fp16 output)
            rl = attnp.tile([128, 512], f32, tag="rl", bufs=2, name="rl")
            nc.vector.reciprocal_approx_fast(out=rl, in_=rlb[:, 0:512])
            nc.vector.tensor_mul(attnT[c][:, jj * 512:(jj + 1) * 512], au, rl)

        # global filler queue: head-1 k, head-2 q/k, head-3 q/k projections.
        # Stage j's rope(j) requires head-j's projection fully consumed, so
        # each stage drains through its threshold before rope at chunk 2.
        from itertools import chain as _chain

        # Fill windows balance: v-proj tail (144 ops) feeds stage 1, head-2
        # projection feeds stage 2, head-3 feeds stage 3, out-proj feeds F.
        # A stage may not pull the NEXT head's ops until its rope has run
        # (single qs_st/ks_st staging buffer).
        VT = (KT - 4) * (17 if zero_bias else 18)   # v-proj tail ops
        fill_q = _chain(vproj_ops(range(4, KT)),
                        proj_ops(2, pool=psm), proj_ops(3, pool=psm))
        consumed = [0]
        limit = [0]

        def pull():
            for _ in range(12):
                if consumed[0] >= limit[0]:
                    return
                f = next(fill_q, None)
                if f is None:
                    return
                f()
                consumed[0] += 1

        def drain_until(n):
            while consumed[0] < n:
                f = next(fill_q, None)
                if f is None:
                    break
                f()
                consumed[0] += 1

        # (head, pre-rope limit, chunk-2 drain, rope head, post limit,
        #  per-chunk pre-drains) -- stage 1's pre-drains guarantee the vs
        # tiles each chunk's au needs have been produced; caps keep the next
        # head's evacuations behind the rope that frees qs_st/ks_st.
        plan = ((0, VT + 136, VT, None, VT + 136, (0, 72, 144, 216)),
                (1, VT + 136, VT + 136, 2, VT + 204, None),
                (2, VT + 272, VT + 272, 3, VT + 272, None))
        for jj, lim, drain_at, rope_j, post, pre in plan:
            if jj + 2 < NHC_:
                load_w(jj + 2)
            limit[0] = lim
            for c in range(CC):
                if pre is not None and pre[c]:
                    drain_until(pre[c])
                attn_chunk(jj, c, pull, diag_first=(c != CC - 1))
                if c == CC - 2:
                    drain_until(drain_at)
                    if rope_j is not None:
                        rope(rope_j)
                    limit[0] = post

        # ---------------- final: head-3 attention + output projection ----------------
        def outproj_chunk(c):
            """Output-projection ops for chunk c (needs attnT[c] complete)."""
            for st in range(4):
                for half in range(2):
                    osb = attnp.tile([128, 1024], f16, tag="osb", bufs=2,
                                     name="osb")
                    for hh in range(2):
                        hc = half * 2 + hh
                        op = psm.tile([128, 512], f32, tag="pp", bufs=2,
                                      name="op")
                        for jj in range(NHC_):
                            yield lambda op=op, c=c, st=st, hc=hc, jj=jj: \
                                nc.tensor.matmul(
                                    op,
                                    lhsT=attnT[c][:, jj * 512 + st * 128:
                                                  jj * 512 + st * 128 + 128],
                                    rhs=wo_sb[:, jj * H_ + hc * 512:
                                              jj * H_ + hc * 512 + 512],
                                    start=(jj == 0), stop=(jj == NHC_ - 1))
                        # alternate PSUM evacuation between vector and scalar
                        if hc % 2 == 0:
                            yield lambda op=op, osb=osb, hh=hh: \
                                nc.vector.tensor_copy(
                                    osb[:, hh * 512:(hh + 1) * 512], op)
                        else:
                            yield lambda op=op, osb=osb, hh=hh: \
                                nc.scalar.activation(
                                    osb[:, hh * 512:(hh + 1) * 512], op,
                                    AF.Copy)

                    def store(osb=osb, c=c, st=st, half=half):
                        row = c * 512 + st * 128
                        nc.sync.dma_start(
                            out=o_d[row: row + 128,
                                    half * 1024:(half + 1) * 1024],
                            in_=osb)
                    yield store

        dq = deque()

        def pull_dq():
            for _ in range(12):
                if not dq:
                    return
                dq.popleft()()

        for c in range(CC):
            attn_chunk(NHC_ - 1, c, pull_dq)
            dq.extend(outproj_chunk(c))
        while dq:
            dq.popleft()()

    nc.compile()
    return nc


def prep_core_inputs(hidden_b, mask_b, Wq, bq, Wk, bk, Wv, bv, Wo, n0, S_, H_,
                     NHC_, cosT, ssT, mdiag):
    """Host-side prep of one core's input map. hidden_b [S,H] f32."""
    T = H_ // 128
    f16 = np.float16

    hT = np.ascontiguousarray(hidden_b.T).reshape(T, 128, S_).astype(f16)

    def w_slices(W):
        # [H, NH, HD] -> [NHC, 128, T*HD]
        out = np.empty((NHC_, 128, T * HD), f16)
        for j in range(NHC_):
            w = W[:, n0 + j, :].reshape(T, 128, HD)         # [t, p, d]
            out[j] = w.transpose(1, 0, 2).reshape(128, T * HD)
        return out

    wq = w_slices(Wq)
    wk = w_slices(Wk)
    # pre-transposed into SBUF layout for contiguous DMA
    wv = np.ascontiguousarray(
        Wv[:, n0:n0 + NHC_, :].reshape(T, 128, NHC_ * HD).transpose(1, 0, 2)
    ).reshape(128, T * NHC_ * HD).astype(f16)
    wo = np.ascontiguousarray(
        Wo[n0:n0 + NHC_].transpose(1, 0, 2)
    ).reshape(128, NHC_ * H_).astype(f16)

    bqT = np.ascontiguousarray(bq[n0:n0 + NHC_].T).astype(np.float32)
    bkT = np.ascontiguousarray(bk[n0:n0 + NHC_].T).astype(np.float32)
    bv4 = bv[n0:n0 + NHC_].reshape(1, NHC_ * HD).astype(f16)

    return {
        "hT": hT, "wq": wq, "wk": wk, "wv": wv, "wo": wo,
        "cosT": cosT, "ssT": ssT, "bqT": bqT, "bkT": bkT, "bv4": bv4,
        "mdiag": mdiag,
    }


def _check_causal_and_diag(mask):
    """Verify the mask is the standard causal mask (shared by all batches)
    and build the shared [128, 4*512] diagonal block in [k, q] layout."""
    S_ = mask.shape[-1]
    m01 = (mask[0, 0] <= 0.5)                      # True where attention allowed
    # diagonal block from chunk 0: rows k 0..511, cols q 0..511, [k, q] layout
    blk = m01[:512, :512].T.copy()                 # [k, q] -> wait: m01 is [q, k]
    # m01[q, k]: allowed = k <= q. Transposed to [k, q]:
    mT = m01.T                                     # [k, q]
    blk = mT[:512, :512]                           # [k, q] diagonal block
    mdiag = np.ascontiguousarray(
        blk.reshape(4, 128, 512).transpose(1, 0, 2).reshape(128, 4 * 512)
    ).astype(np.float16)
    # verify causal structure cheaply: the full mask must equal k <= q
    q_idx = np.arange(S_)
    expect_rows = [0, 1, 511, 512, 1000, 2047]
    for r in expect_rows:
        if not np.array_equal(m01[r], q_idx <= r):
            raise ValueError("mask is not the standard causal mask")
    for b in range(mask.shape[0]):
        if not np.array_equal((mask[b, 0] <= 0.5), m01):
            raise ValueError("mask differs across batches")
    return mdiag


def kernel(hidden_states, mask, Wq, bq, Wk, bk, Wv, bv, Wo, bo):
    global LAST_RESULTS
    from concourse.bass_utils import run_bass_kernel_spmd

    hidden_states = np.asarray(hidden_states, dtype=np.float32)
    mask = np.asarray(mask, dtype=np.float32)
    Wq, bq = np.asarray(Wq, np.float32), np.asarray(bq, np.float32)
    Wk, bk = np.asarray(Wk, np.float32), np.asarray(bk, np.float32)
    Wv, bv = np.asarray(Wv, np.float32), np.asarray(bv, np.float32)
    Wo, bo = np.asarray(Wo, np.float32), np.asarray(bo, np.float32)

    cosT, ssT = _rope_tables(S)
    mdiag = _check_causal_and_diag(mask)

    in_maps = []
    for core in range(N_CORES):
        b = core // HGRID
        n0 = (core % HGRID) * NHC
        in_maps.append(prep_core_inputs(
            hidden_states[b], mask[b, 0], Wq, bq, Wk, bk, Wv, bv, Wo,
            n0, S, H, NHC, cosT, ssT, mdiag))

    zb = not (bq.any() or bk.any() or bv.any())
    key = (S, H, NHC, zb)
    if key not in _CACHE:
        _CACHE[key] = build_program(S, H, NHC, zero_bias=zb)
    nc = _CACHE[key]

    res = run_bass_kernel_spmd(nc, in_maps, core_ids=list(range(N_CORES)))
    LAST_RESULTS = res

    out = np.zeros((B, S, H), np.float32)
    for core in range(N_CORES):
        out[core // HGRID] += res.results[core]["o"].astype(np.float32)
    out += bo[None, None, :]
    return out


# revision 40
# speedup vs baseline: 1.1398x; 1.1398x over previous
"""Trainium2 Bass kernel for multi-head self-attention with RoPE.

Sharding: 8 cores = 2 (batch) x 4 (head groups of 4 heads).
Each core computes its batch's attention for its 4 heads plus the
(row-sharded) output projection partial sum; the host adds the 4 head-group
partials per batch and the output bias.

Schedule (single Tile program, engines self-synchronize):
  S0:      head-0 q/k projection t-outer across 8 PSUM banks so the PE
           consumes hidden-state tiles as the DMAs land; then v projection.
  stage j: head j's q/k projection matmuls are interleaved between head
           j-1's score matmuls as PE filler while the scalar engine
           computes exp() of the score tiles; per (chunk, head) the softmax
           denominator is a vector-engine tree-sum of the exp tiles plus a
           single ones-matmul broadcast, inverted with vector reciprocal
           (no Ln -> single ACT table, no table thrash).
  final:   head-3 attention chunk-by-chunk with the previous chunk's
           output-projection matmuls as PE filler.

The causal mask multiply uses one shared [128, 4*512] diagonal block
(identical for every query chunk) applied as a single tensor_mul per
(chunk, head). Output is stored fp16; host adds partials in fp32.
"""

import sys
import types
from collections import deque

import numpy as np

sys.path.insert(0, "/opt/trn_rl_repo")

# The axon boot registers its NTFF-profiling hook via antenv.axon_hooks; some
# images lack that module, which silently disables tracing. Provide it.
if "antenv.axon_hooks" not in sys.modules:
    try:
        import antenv.axon_hooks  # noqa: F401
    except ImportError:
        try:
            import antenv

            _m = types.ModuleType("antenv.axon_hooks")
            _m._hook = None
            _m.set_axon_ntff_profile_hook = lambda h: setattr(_m, "_hook", h)
            _m.get_axon_ntff_profile_hook = lambda: _m._hook
            sys.modules["antenv.axon_hooks"] = _m
            antenv.axon_hooks = _m
        except ImportError:
            pass

B, S, H, NH, HD = 2, 2048, 2048, 16, 128
ROPE_THETA = 10000.0
N_CORES = 8
HGRID = 4            # head-group shards
NHC = NH // HGRID    # heads per core

LAST_RESULTS = None  # test harness introspection
_CACHE = {}


def _rope_tables(S_, dtype=np.float16):
    # transposed rope tables [HD, S]; ss has rotate-half sign folded in:
    # rope(x)[d, s] = x[d, s]*cosT[d, s] + x[(d+64)%128, s]*ss[d, s]
    inv = 1.0 / (ROPE_THETA ** (np.arange(0, HD, 2, dtype=np.float64) / HD))
    t = np.arange(S_, dtype=np.float64)
    fr = np.outer(t, inv)                          # [S, HD/2]
    emb = np.concatenate([fr, fr], axis=1)         # [S, HD]
    cosT = np.cos(emb).T.astype(np.float32)        # [HD, S]
    ss = np.sin(emb).T.astype(np.float32)
    ss[: HD // 2] *= -1.0
    return cosT.astype(dtype), ss.astype(dtype)


def build_program(S_, H_, NHC_, zero_bias=False):
    """Build + compile the per-core SPMD bass program (causal schedule)."""
    from contextlib import ExitStack

    import concourse.mybir as mybir
    import concourse.tile as tile
    from concourse import bacc

    f16 = mybir.dt.float16
    f32 = mybir.dt.float32
    AF = mybir.ActivationFunctionType

    T = H_ // 128       # hidden contraction tiles
    KT = S_ // 128      # key/seq tiles
    CC = S_ // 512      # query chunks
    HC = H_ // 512      # output hidden chunks
    SCALE = 1.0 / float(np.sqrt(HD))

    nc = bacc.Bacc("TRN2", target_bir_lowering=False, debug=False)

    hT_d = nc.dram_tensor("hT", [T, 128, S_], f16, kind="ExternalInput").ap()
    wq_d = nc.dram_tensor("wq", [NHC_, 128, T * HD], f16, kind="ExternalInput").ap()
    wk_d = nc.dram_tensor("wk", [NHC_, 128, T * HD], f16, kind="ExternalInput").ap()
    # wv/wo are pre-transposed host-side into SBUF layout (contiguous DMA)
    wv_d = nc.dram_tensor("wv", [128, T * NHC_ * HD], f16, kind="ExternalInput").ap()
    wo_d = nc.dram_tensor("wo", [128, NHC_ * H_], f16, kind="ExternalInput").ap()
    cos_d = nc.dram_tensor("cosT", [128, S_], f16, kind="ExternalInput").ap()
    ss_d = nc.dram_tensor("ssT", [128, S_], f16, kind="ExternalInput").ap()
    bq_d = nc.dram_tensor("bqT", [128, NHC_], f32, kind="ExternalInput").ap()
    bk_d = nc.dram_tensor("bkT", [128, NHC_], f32, kind="ExternalInput").ap()
    bv_d = nc.dram_tensor("bv4", [1, NHC_ * HD], f16, kind="ExternalInput").ap()
    md_d = nc.dram_tensor("mdiag", [128, 4 * 512], f16, kind="ExternalInput").ap()
    o_d = nc.dram_tensor("o", [S_, H_], f16, kind="ExternalOutput").ap()

    with ExitStack() as ctx:
        tc = ctx.enter_context(tile.TileContext(nc))
        persist = ctx.enter_context(tc.tile_pool(name="persist", bufs=1))

        # qr/kr 2-deep rings: stage j writes ring[j % 2]; head j's attention
        # reads it during stage j+1.
        qr = [persist.tile([128, S_], f16, name=f"qr{r}") for r in range(2)]
        kr = [persist.tile([128, S_], f16, name=f"kr{r}") for r in range(2)]
        vs = persist.tile([128, KT * NHC_ * HD], f16, name="vs")
        wo_sb = persist.tile([128, NHC_ * H_], f16, name="wo_sb")
        attnT = [persist.tile([128, NHC_ * 512], f16, name=f"attnT{c}")
                 for c in range(CC)]
        cos_sb = persist.tile([128, S_], f16, name="cos_sb")
        ss_sb = persist.tile([128, S_], f16, name="ss_sb")
        md_sb = persist.tile([128, 4 * 512], f16, name="md_sb")
        ones_sb = persist.tile([128, 128], f16, name="ones_sb")
        ones1 = persist.tile([1, 128], f16, name="ones1")
        bv_sb = persist.tile([1, NHC_ * HD], f16, name="bv_sb")
        bq_sb = persist.tile([128, NHC_], f32, name="bq_sb")
        bk_sb = persist.tile([128, NHC_], f32, name="bk_sb")
        # single staging buffer for pre-rope q/k (vector-queue order makes
        # reuse across stages safe: rope(j) reads are issued before stage
        # j+1's evac writes on the same queue)
        qs_st = persist.tile([128, S_], f16, name="qs_st")
        ks_st = persist.tile([128, S_], f16, name="ks_st")

        nc.vector.memset(ones_sb, 1.0)
        nc.vector.memset(ones1, 1.0)

        # per-head q/k weight tiles, 4-deep rotation (j and j+1 in flight)
        wpool = ctx.enter_context(tc.tile_pool(name="wpool", bufs=1))

        # ---- DMA issue (3 dynamic queues: sync, gpsimd, scalar) ----
        # Startup transfers are quartered (128KB) and dealt round-robin to
        # the three queues in PE-need order, so the first matmul's inputs
        # land within ~1us of queue start and the t-loop streams.
        hT_pool = ctx.enter_context(tc.tile_pool(name="hTp", bufs=1))
        # hT_sb[t][g]: quarter tiles [128, 512] (g = column group)
        hT_sb = [[hT_pool.tile([128, 512], f16, name=f"hT{t}_{g}")
                  for g in range(4)] for t in range(T)]
        # head-0 q/k weights quartered: w0x[g] holds t-slices 4g..4g+3
        # all q/k weights stored as 4 quarter-tiles [128, 512] per (head,
        # q/k), sharing one 16-deep rotation (two heads in flight)
        wst = {}

        def walloc(j, nm):
            wst[(j, nm)] = [wpool.tile([128, 512], f16, tag="w", bufs=16,
                                       name=f"w{j}{nm}_{g}") for g in range(4)]

        walloc(0, "q")
        walloc(0, "k")
        walloc(1, "q")
        walloc(1, "k")

        def wslice(j, nm, t):
            return wst[(j, nm)][t // 4][:, (t % 4) * HD:(t % 4 + 1) * HD]

        # small tensors ride early prio slots: the head-0 PSUM evacuations
        # need the biases, rope(0) needs cos/ss, well before the bulk
        # stream finishes
        prio = []   # (out_tile, dram_ap) in PE-need order
        for t in range(T):
            if t < 4:
                prio.append((wst[(0, "q")][t], wq_d[0][:, t * 512:(t + 1) * 512]))
                prio.append((wst[(0, "k")][t], wk_d[0][:, t * 512:(t + 1) * 512]))
            for g in range(4):
                prio.append((hT_sb[t][g], hT_d[t][:, g * 512:(g + 1) * 512]))
            if t == 0:
                prio.append((bq_sb, bq_d))
                prio.append((bk_sb, bk_d))
                prio.append((bv_sb, bv_d))
            if t == 11:     # head-1 weights: needed only after the t-loop
                for g in range(4):
                    prio.append((wst[(1, "q")][g],
                                 wq_d[1][:, g * 512:(g + 1) * 512]))
            if t == 13:
                for g in range(4):
                    prio.append((wst[(1, "k")][g],
                                 wk_d[1][:, g * 512:(g + 1) * 512]))
        # post-hT tail in need order: rope tables (rope(0), ~t-loop end),
        # then wv (v-front), then the mask block (first exp, later still)
        prio.append((cos_sb, cos_d))
        prio.append((ss_sb, ss_d))
        dma_eng = [nc.gpsimd, nc.scalar, nc.sync]
        for i, (out_t, in_ap) in enumerate(prio):
            dma_eng[i % 3].dma_start(out=out_t, in_=in_ap)

        ropep = ctx.enter_context(tc.tile_pool(name="ropep", bufs=1))
        wvp = ctx.enter_context(tc.tile_pool(name="wvp", bufs=1))
        wv_sb = wvp.tile([128, T * NHC_ * HD], f16, name="wv_sb")
        for g in range(4):
            sl = slice(g * T * NHC_ * HD // 4, (g + 1) * T * NHC_ * HD // 4)
            dma_eng[g % 3].dma_start(out=wv_sb[:, sl], in_=wv_d[:, sl])
        nc.sync.dma_start(out=md_sb, in_=md_d)

        def load_w(j):
            for nm, w_d0 in (("q", wq_d), ("k", wk_d)):
                walloc(j, nm)
                for g in range(4):
                    nc.sync.dma_start(out=wst[(j, nm)][g],
                                      in_=w_d0[j][:, g * 512:(g + 1) * 512])

        for g in range(4):
            sl = slice(g * NHC_ * H_ // 4, (g + 1) * NHC_ * H_ // 4)
            nc.scalar.dma_start(out=wo_sb[:, sl], in_=wo_d[:, sl])

        # attnp/psm are created after the wv pool closes (SBUF/PSUM reuse);
        # attn_chunk binds them late, first use is after v-projection.

        def rope(j):
            """rope(qs_st/ks_st) -> qr/kr ring j%2 (vector + gpsimd shifts)."""
            r = j % 2
            for src, dst in ((qs_st, qr[r]), (ks_st, kr[r])):
                sh = ropep.tile([128, S_], f16, tag="sh", bufs=1, name="sh")
                acc = ropep.tile([128, S_], f16, tag="racc", bufs=1, name="racc")
                nc.gpsimd.dma_start(out=sh[0:64], in_=src[64:128])
                nc.gpsimd.dma_start(out=sh[64:128], in_=src[0:64])
                nc.vector.tensor_mul(acc, src, cos_sb)
                nc.vector.tensor_mul(dst, sh, ss_sb)
                nc.vector.tensor_add(dst, acc, dst)

        def proj_ops(j, parts=("q", "k"), pool=None):
            """PE-filler closures for head-j q/k projection (+ vector evac)."""
            for nm in parts:
                dstt = qs_st if nm == "q" else ks_st
                b_sb = bq_sb if nm == "q" else bk_sb
                for c in range(CC):
                    pp = pool.tile([128, 512], f32, tag="pp", bufs=2, name="pp")
                    for t in range(T):
                        yield lambda pp=pp, j=j, nm=nm, c=c, t=t: nc.tensor.matmul(
                            pp, lhsT=wslice(j, nm, t), rhs=hT_sb[t][c],
                            start=(t == 0), stop=(t == T - 1))
                    yield lambda pp=pp, dstt=dstt, c=c, b_sb=b_sb, j=j: \
                        nc.vector.tensor_scalar_add(
                            dstt[:, c * 512:(c + 1) * 512], pp, b_sb[:, j:j + 1])

        # ---------------- S0: head-0 q/k projection, t-outer ----------------
        with tc.tile_pool(name="ps0", bufs=1, space="PSUM") as ps0:
            ppq = [ps0.tile([128, 512], f32, tag="pj0", bufs=8, name=f"ppq{c}")
                   for c in range(CC)]
            ppk = [ps0.tile([128, 512], f32, tag="pj0", bufs=8, name=f"ppk{c}")
                   for c in range(CC)]
            for t in range(T):
                wq0 = wst[(0, "q")][t // 4][:, (t % 4) * HD:(t % 4 + 1) * HD]
                wk0 = wst[(0, "k")][t // 4][:, (t % 4) * HD:(t % 4 + 1) * HD]
                for c in range(CC):
                    nc.tensor.matmul(
                        ppq[c], lhsT=wq0, rhs=hT_sb[t][c],
                        start=(t == 0), stop=(t == T - 1))
                    nc.tensor.matmul(
                        ppk[c], lhsT=wk0, rhs=hT_sb[t][c],
                        start=(t == 0), stop=(t == T - 1))
            for c in range(CC):
                nc.vector.tensor_scalar_add(
                    qs_st[:, c * 512:(c + 1) * 512], ppq[c], bq_sb[:, 0:1])
                nc.vector.tensor_scalar_add(
                    ks_st[:, c * 512:(c + 1) * 512], ppk[c], bk_sb[:, 0:1])

        # rope(0) runs as soon as head-0 evacs land, freeing qs_st/ks_st
        # for the later heads' projections
        rope(0)

        psm = ctx.enter_context(tc.tile_pool(name="psm", bufs=1, space="PSUM"))

        def vproj_ops(sts):
            """v-projection ops for the given s-tiles (16 mm + bias + copy)."""
            for st in sts:
                vp = psm.tile([128, NHC_ * HD], f32, tag="pp", bufs=2,
                              name="vp")
                for t in range(T):
                    yield lambda vp=vp, st=st, t=t: nc.tensor.matmul(
                        vp,
                        lhsT=hT_sb[t][st // 4][:, (st % 4) * 128:
                                               (st % 4) * 128 + 128],
                        rhs=wv_sb[:, t * NHC_ * HD:(t + 1) * NHC_ * HD],
                        start=(t == 0),
                        stop=(zero_bias and t == T - 1))
                if not zero_bias:
                    yield lambda vp=vp: nc.tensor.matmul(
                        vp, lhsT=ones1, rhs=bv_sb, start=False, stop=True)
                yield lambda vp=vp, st=st: nc.vector.tensor_copy(
                    vs[:, st * NHC_ * HD:(st + 1) * NHC_ * HD], vp)

        # ---------------- S0 tail: head-1 projection + rope + v-proj front --
        # the input stream (hT+wv) is still landing; head-1's projection and
        # the front half of the v-projection keep the PE busy through it
        for f in proj_ops(1, ("q", "k"), pool=psm):
            f()
        rope(1)
        for f in vproj_ops(range(0, 4)):
            f()

        attnp = ctx.enter_context(tc.tile_pool(name="attnp", bufs=1))

        def attn_chunk(jj, c, pull, diag_first=True):
            """Head jj's chunk-c scores/exp/au/normalize; pull() issues
            PE-filler ops between score pairs. The au accumulation is
            interleaved per pair so each exp tile dies immediately
            (expT is only [128, 1024]); the softmax denominator is a
            running vector sum into expsum."""
            rr = jj % 2
            inc = 4 * (c + 1)           # causal: k-tiles 0..4c+3
            # diag_first: masked diagonal pairs first so the chunk tail has
            # no mask dependency; diag-last when the vector queue is busy
            # early in the chunk (rope overlap)
            if diag_first:
                order = [inc - 4, inc - 2] + list(range(0, inc - 4, 2))
            else:
                order = list(range(0, inc, 2))
            au = psm.tile([128, 512], f32, tag="au", bufs=2, name="au")
            expsum = attnp.tile([128, 512], f16, tag="expsum", bufs=2,
                                name="expsum")
            prev = None
            first_pair = order[0]

            def au_and_sum(q0, qexpT):
                for kk in (0, 1):
                    kt = q0 + kk
                    nc.tensor.matmul(
                        au,
                        lhsT=vs[:, (kt * NHC_ + jj) * HD:
                                (kt * NHC_ + jj + 1) * HD],
                        rhs=qexpT[:, kk * 512:(kk + 1) * 512],
                        start=(q0 == first_pair and kk == 0),
                        stop=(q0 == order[-1] and kk == 1))
                if q0 == first_pair:
                    nc.vector.tensor_add(
                        expsum, qexpT[:, 0:512], qexpT[:, 512:1024])
                else:
                    nc.vector.tensor_add(expsum, expsum, qexpT[:, 0:512])
                    nc.vector.tensor_add(expsum, expsum, qexpT[:, 512:1024])

            for p0 in order:
                scp = psm.tile([128, 1024], f32, tag="scp", bufs=2, name="scp")
                for kk in (0, 1):
                    kt = p0 + kk
                    nc.tensor.matmul(
                        scp[:, kk * 512:(kk + 1) * 512],
                        lhsT=kr[rr][:, kt * 128:(kt + 1) * 128],
                        rhs=qr[rr][:, c * 512:(c + 1) * 512],
                        start=True, stop=True)
                expT = attnp.tile([128, 1024], f16, tag="expT", bufs=3,
                                  name="expT")
                # fused scale+exp over the pair
                nc.scalar.activation(expT, scp, AF.Exp, scale=SCALE)
                if p0 >= inc - 4:
                    # multiplicative causal mask (diag tiles p0-(inc-4), +1)
                    i0 = (p0 - (inc - 4)) * 512
                    nc.vector.tensor_mul(expT, expT, md_sb[:, i0:i0 + 1024])
                # au accumulation for the previous pair (its exp is ready)
                if prev is not None:
                    au_and_sum(*prev)
                prev = (p0, expT)
                pull()
            au_and_sum(*prev)
            # denominator broadcast into a scp-tag buffer (au keeps 2-deep
            # rotation so the divide latency is off the PE critical path)
            rlb = psm.tile([128, 1024], f32, tag="scp", bufs=2, name="rlb")
            nc.tensor.matmul(rlb[:, 0:512], lhsT=ones_sb, rhs=expsum,
                             start=True, stop=True)
            # 1/l at ~18 bits via one custom-DVE op (plenty for fp16 output)
            rl = attnp.tile([128, 512], f32, tag="rl", bufs=2, name="rl")
            nc.vector.reciprocal_approx_fast(out=rl, in_=rlb[:, 0:512])
            nc.vector.tensor_mul(attnT[c][:, jj * 512:(jj + 1) * 512], au, rl)

        # global filler queue: head-1 k, head-2 q/k, head-3 q/k projections.
        # Stage j's rope(j) requires head-j's projection fully consumed, so
        # each stage drains through its threshold before rope at chunk 2.
        from itertools import chain as _chain

        # Fill windows balance: v-proj tail (144 ops) feeds stage 1, head-2
        # projection feeds stage 2, head-3 feeds stage 3, out-proj feeds F.
        # A stage may not pull the NEXT head's ops until its rope has run
        # (single qs_st/ks_st staging buffer).
        VT = (KT - 4) * (17 if zero_bias else 18)   # v-proj tail ops
        fill_q = _chain(vproj_ops(range(4, KT)),
                        proj_ops(2, pool=psm), proj_ops(3, pool=psm))
        consumed = [0]
        limit = [0]

        def pull():
            for _ in range(12):
                if consumed[0] >= limit[0]:
                    return
                f = next(fill_q, None)
                if f is None:
                    return
                f()
                consumed[0] += 1

        def drain_until(n):
            while consumed[0] < n:
                f = next(fill_q, None)
                if f is None:
                    break
                f()
                consumed[0] += 1

        # (head, pre-rope limit, chunk-2 drain, rope head, post limit,
        #  per-chunk pre-drains) -- stage 1's pre-drains guarantee the vs
        # tiles each chunk's au needs have been produced; caps keep the next
        # head's evacuations behind the rope that frees qs_st/ks_st.
        plan = ((0, VT + 136, VT, None, VT + 136, (0, 72, 144, 216)),
                (1, VT + 136, VT + 136, 2, VT + 204, None),
                (2, VT + 272, VT + 272, 3, VT + 272, None))
        for jj, lim, drain_at, rope_j, post, pre in plan:
            if jj + 2 < NHC_:
                load_w(jj + 2)
            limit[0] = lim
            for c in range(CC):
                if pre is not None and pre[c]:
                    drain_until(pre[c])
                attn_chunk(jj, c, pull, diag_first=(c != CC - 1))
                if c == CC - 2:
                    drain_until(drain_at)
                    if rope_j is not None:
                        rope(rope_j)
                    limit[0] = post

        # ---------------- final: head-3 attention + output projection ----------------
        def outproj_chunk(c):
            """Output-projection ops for chunk c (needs attnT[c] complete)."""
            for st in range(4):
                for half in range(2):
                    osb = attnp.tile([128, 1024], f16, tag="osb", bufs=2,
                                     name="osb")
                    for hh in range(2):
                        hc = half * 2 + hh
                        op = psm.tile([128, 512], f32, tag="pp", bufs=2,
                                      name="op")
                        for jj in range(NHC_):
                            yield lambda op=op, c=c, st=st, hc=hc, jj=jj: \
                                nc.tensor.matmul(
                                    op,
                                    lhsT=attnT[c][:, jj * 512 + st * 128:
                                                  jj * 512 + st * 128 + 128],
                                    rhs=wo_sb[:, jj * H_ + hc * 512:
                                              jj * H_ + hc * 512 + 512],
                                    start=(jj == 0), stop=(jj == NHC_ - 1))
                        # alternate PSUM evacuation between vector and scalar
                        if hc % 2 == 0:
                            yield lambda op=op, osb=osb, hh=hh: \
                                nc.vector.tensor_copy(
                                    osb[:, hh * 512:(hh + 1) * 512], op)
                        else:
                            yield lambda op=op, osb=osb, hh=hh: \
                                nc.scalar.activation(
                                    osb[:, hh * 512:(hh + 1) * 512], op,
                                    AF.Copy)

                    def store(osb=osb, c=c, st=st, half=half):
                        row = c * 512 + st * 128
                        nc.sync.dma_start(
                            out=o_d[row: row + 128,
                                    half * 1024:(half + 1) * 1024],
                            in_=osb)
                    yield store

        dq = deque()

        def pull_dq():
            for _ in range(12):
                if not dq:
                    return
                dq.popleft()()

        for c in range(CC):
            attn_chunk(NHC_ - 1, c, pull_dq)
            dq.extend(outproj_chunk(c))
        while dq:
            dq.popleft()()

    nc.compile()
    return nc


def prep_core_inputs(hidden_b, mask_b, Wq, bq, Wk, bk, Wv, bv, Wo, n0, S_, H_,
                     NHC_, cosT, ssT, mdiag):
    """Host-side prep of one core's input map. hidden_b [S,H] f32."""
    T = H_ // 128
    f16 = np.float16

    hT = np.ascontiguousarray(hidden_b.T).reshape(T, 128, S_).astype(f16)

    def w_slices(W):
        # [H, NH, HD] -> [NHC, 128, T*HD]
        out = np.empty((NHC_, 128, T * HD), f16)
        for j in range(NHC_):
            w = W[:, n0 + j, :].reshape(T, 128, HD)         # [t, p, d]
            out[j] = w.transpose(1, 0, 2).reshape(128, T * HD)
        return out

    wq = w_slices(Wq)
    wk = w_slices(Wk)
    # pre-transposed into SBUF layout for contiguous DMA
    wv = np.ascontiguousarray(
        Wv[:, n0:n0 + NHC_, :].reshape(T, 128, NHC_ * HD).transpose(1, 0, 2)
    ).reshape(128, T * NHC_ * HD).astype(f16)
    wo = np.ascontiguousarray(
        Wo[n0:n0 + NHC_].transpose(1, 0, 2)
    ).reshape(128, NHC_ * H_).astype(f16)

    bqT = np.ascontiguousarray(bq[n0:n0 + NHC_].T).astype(np.float32)
    bkT = np.ascontiguousarray(bk[n0:n0 + NHC_].T).astype(np.float32)
    bv4 = bv[n0:n0 + NHC_].reshape(1, NHC_ * HD).astype(f16)

    return {
        "hT": hT, "wq": wq, "wk": wk, "wv": wv, "wo": wo,
        "cosT": cosT, "ssT": ssT, "bqT": bqT, "bkT": bkT, "bv4": bv4,
        "mdiag": mdiag,
    }


def _check_causal_and_diag(mask):
    """Verify the mask is the standard causal mask (shared by all batches)
    and build the shared [128, 4*512] diagonal block in [k, q] layout."""
    S_ = mask.shape[-1]
    m01 = (mask[0, 0] <= 0.5)                      # True where attention allowed
    # diagonal block from chunk 0: rows k 0..511, cols q 0..511, [k, q] layout
    blk = m01[:512, :512].T.copy()                 # [k, q] -> wait: m01 is [q, k]
    # m01[q, k]: allowed = k <= q. Transposed to [k, q]:
    mT = m01.T                                     # [k, q]
    blk = mT[:512, :512]                           # [k, q] diagonal block
    mdiag = np.ascontiguousarray(
        blk.reshape(4, 128, 512).transpose(1, 0, 2).reshape(128, 4 * 512)
    ).astype(np.float16)
    # verify causal structure cheaply: the full mask must equal k <= q
    q_idx = np.arange(S_)
    expect_rows = [0, 1, 511, 512, 1000, 2047]
    for r in expect_rows:
        if not np.array_equal(m01[r], q_idx <= r):
            raise ValueError("mask is not the standard causal mask")
    for b in range(mask.shape[0]):
        if not np.array_equal((mask[b, 0] <= 0.5), m01):
            raise ValueError("mask differs across batches")
    return mdiag


def kernel(hidden_states, mask, Wq, bq, Wk, bk, Wv, bv, Wo, bo):
    global LAST_RESULTS
    from concourse.bass_utils import run_bass_kernel_spmd

    hidden_states = np.asarray(hidden_states, dtype=np.float32)
    mask = np.asarray(mask, dtype=np.float32)
    Wq, bq = np.asarray(Wq, np.float32), np.asarray(bq, np.float32)
    Wk, bk = np.asarray(Wk, np.float32), np.asarray(bk, np.float32)
    Wv, bv = np.asarray(Wv, np.float32), np.asarray(bv, np.float32)
    Wo, bo = np.asarray(Wo, np.float32), np.asarray(bo, np.float32)

    cosT, ssT = _rope_tables(S)
    mdiag = _check_causal_and_diag(mask)

    in_maps = []
    for core in range(N_CORES):
        b = core // HGRID
        n0 = (core % HGRID) * NHC
        in_maps.append(prep_core_inputs(
            hidden_states[b], mask[b, 0], Wq, bq, Wk, bk, Wv, bv, Wo,
            n0, S, H, NHC, cosT, ssT, mdiag))

    zb = not (bq.any() or bk.any() or bv.any())
    key = (S, H, NHC, zb)
    if key not in _CACHE:
        _CACHE[key] = build_program(S, H, NHC, zero_bias=zb)
    nc = _CACHE[key]

    res = run_bass_kernel_spmd(nc, in_maps, core_ids=list(range(N_CORES)))
    LAST_RESULTS = res

    out = np.zeros((B, S, H), np.float32)
    for core in range(N_CORES):
        out[core // HGRID] += res.results[core]["o"].astype(np.float32)
    out += bo[None, None, :]
    return out


# revision 42
# speedup vs baseline: 1.1707x; 1.0272x over previous
"""Trainium2 Bass kernel for multi-head self-attention with RoPE.

Sharding: 8 cores = 2 (batch) x 4 (head groups of 4 heads).
Each core computes its batch's attention for its 4 heads plus the
(row-sharded) output projection partial sum; the host adds the 4 head-group
partials per batch and the output bias.

Schedule (single Tile program, engines self-synchronize):
  S0:      head-0 q/k projection t-outer across 8 PSUM banks so the PE
           consumes hidden-state tiles as the DMAs land; then v projection.
  stage j: head j's q/k projection matmuls are interleaved between head
           j-1's score matmuls as PE filler while the scalar engine
           computes exp() of the score tiles; per (chunk, head) the softmax
           denominator is a vector-engine tree-sum of the exp tiles plus a
           single ones-matmul broadcast, inverted with vector reciprocal
           (no Ln -> single ACT table, no table thrash).
  final:   head-3 attention chunk-by-chunk with the previous chunk's
           output-projection matmuls as PE filler.

The causal mask multiply uses one shared [128, 4*512] diagonal block
(identical for every query chunk) applied as a single tensor_mul per
(chunk, head). Output is stored fp16; host adds partials in fp32.
"""

import sys
import types
from collections import deque

import numpy as np

sys.path.insert(0, "/opt/trn_rl_repo")

# The axon boot registers its NTFF-profiling hook via antenv.axon_hooks; some
# images lack that module, which silently disables tracing. Provide it.
if "antenv.axon_hooks" not in sys.modules:
    try:
        import antenv.axon_hooks  # noqa: F401
    except ImportError:
        try:
            import antenv

            _m = types.ModuleType("antenv.axon_hooks")
            _m._hook = None
            _m.set_axon_ntff_profile_hook = lambda h: setattr(_m, "_hook", h)
            _m.get_axon_ntff_profile_hook = lambda: _m._hook
            sys.modules["antenv.axon_hooks"] = _m
            antenv.axon_hooks = _m
        except ImportError:
            pass

B, S, H, NH, HD = 2, 2048, 2048, 16, 128
ROPE_THETA = 10000.0
N_CORES = 8
HGRID = 4            # head-group shards
NHC = NH // HGRID    # heads per core

LAST_RESULTS = None  # test harness introspection
_CACHE = {}


def _rope_tables(S_, dtype=np.float16):
    # transposed rope tables [HD, S]; ss has rotate-half sign folded in:
    # rope(x)[d, s] = x[d, s]*cosT[d, s] + x[(d+64)%128, s]*ss[d, s]
    inv = 1.0 / (ROPE_THETA ** (np.arange(0, HD, 2, dtype=np.float64) / HD))
    t = np.arange(S_, dtype=np.float64)
    fr = np.outer(t, inv)                          # [S, HD/2]
    emb = np.concatenate([fr, fr], axis=1)         # [S, HD]
    cosT = np.cos(emb).T.astype(np.float32)        # [HD, S]
    ss = np.sin(emb).T.astype(np.float32)
    ss[: HD // 2] *= -1.0
    return cosT.astype(dtype), ss.astype(dtype)


def build_program(S_, H_, NHC_, zero_bias=False):
    """Build + compile the per-core SPMD bass program (causal schedule)."""
    from contextlib import ExitStack

    import concourse.mybir as mybir
    import concourse.tile as tile
    from concourse import bacc

    f16 = mybir.dt.float16
    f32 = mybir.dt.float32
    AF = mybir.ActivationFunctionType

    T = H_ // 128       # hidden contraction tiles
    KT = S_ // 128      # key/seq tiles
    CC = S_ // 512      # query chunks
    HC = H_ // 512      # output hidden chunks
    SCALE = 1.0 / float(np.sqrt(HD))

    nc = bacc.Bacc("TRN2", target_bir_lowering=False, debug=False)

    hT_d = nc.dram_tensor("hT", [T, 128, S_], f16, kind="ExternalInput").ap()
    wq_d = nc.dram_tensor("wq", [NHC_, 128, T * HD], f16, kind="ExternalInput").ap()
    wk_d = nc.dram_tensor("wk", [NHC_, 128, T * HD], f16, kind="ExternalInput").ap()
    # wv/wo are pre-transposed host-side into SBUF layout (contiguous DMA)
    wv_d = nc.dram_tensor("wv", [128, T * NHC_ * HD], f16, kind="ExternalInput").ap()
    wo_d = nc.dram_tensor("wo", [128, NHC_ * H_], f16, kind="ExternalInput").ap()
    cos_d = nc.dram_tensor("cosT", [128, S_], f16, kind="ExternalInput").ap()
    ss_d = nc.dram_tensor("ssT", [128, S_], f16, kind="ExternalInput").ap()
    bq_d = nc.dram_tensor("bqT", [128, NHC_], f32, kind="ExternalInput").ap()
    bk_d = nc.dram_tensor("bkT", [128, NHC_], f32, kind="ExternalInput").ap()
    bv_d = nc.dram_tensor("bv4", [1, NHC_ * HD], f16, kind="ExternalInput").ap()
    md_d = nc.dram_tensor("mdiag", [128, 4 * 512], f16, kind="ExternalInput").ap()
    o_d = nc.dram_tensor("o", [S_, H_], f16, kind="ExternalOutput").ap()

    with ExitStack() as ctx:
        tc = ctx.enter_context(tile.TileContext(nc))
        persist = ctx.enter_context(tc.tile_pool(name="persist", bufs=1))

        # qr/kr 2-deep rings: stage j writes ring[j % 2]; head j's attention
        # reads it during stage j+1.
        qr = [persist.tile([128, S_], f16, name=f"qr{r}") for r in range(2)]
        kr = [persist.tile([128, S_], f16, name=f"kr{r}") for r in range(2)]
        vs = persist.tile([128, KT * NHC_ * HD], f16, name="vs")
        wo_sb = persist.tile([128, NHC_ * H_], f16, name="wo_sb")
        attnT = [persist.tile([128, NHC_ * 512], f16, name=f"attnT{c}")
                 for c in range(CC)]
        cos_sb = persist.tile([128, S_], f16, name="cos_sb")
        ss_sb = persist.tile([128, S_], f16, name="ss_sb")
        md_sb = persist.tile([128, 4 * 512], f16, name="md_sb")
        ones_sb = persist.tile([128, 128], f16, name="ones_sb")
        ones1 = persist.tile([1, 128], f16, name="ones1")
        bv_sb = persist.tile([1, NHC_ * HD], f16, name="bv_sb")
        bq_sb = persist.tile([128, NHC_], f32, name="bq_sb")
        bk_sb = persist.tile([128, NHC_], f32, name="bk_sb")
        # single staging buffer for pre-rope q/k (vector-queue order makes
        # reuse across stages safe: rope(j) reads are issued before stage
        # j+1's evac writes on the same queue)
        qs_st = persist.tile([128, S_], f16, name="qs_st")
        ks_st = persist.tile([128, S_], f16, name="ks_st")

        nc.vector.memset(ones_sb, 1.0)
        nc.vector.memset(ones1, 1.0)

        # per-head q/k weight tiles, 4-deep rotation (j and j+1 in flight)
        wpool = ctx.enter_context(tc.tile_pool(name="wpool", bufs=1))

        # ---- DMA issue (3 dynamic queues: sync, gpsimd, scalar) ----
        # Startup transfers are quartered (128KB) and dealt round-robin to
        # the three queues in PE-need order, so the first matmul's inputs
        # land within ~1us of queue start and the t-loop streams.
        hT_pool = ctx.enter_context(tc.tile_pool(name="hTp", bufs=1))
        # hT_sb[t][g]: quarter tiles [128, 512] (g = column group)
        hT_sb = [[hT_pool.tile([128, 512], f16, name=f"hT{t}_{g}")
                  for g in range(4)] for t in range(T)]
        # head-0 q/k weights quartered: w0x[g] holds t-slices 4g..4g+3
        # all q/k weights stored as 4 quarter-tiles [128, 512] per (head,
        # q/k), sharing one 16-deep rotation (two heads in flight)
        wst = {}

        def walloc(j, nm):
            wst[(j, nm)] = [wpool.tile([128, 512], f16, tag="w", bufs=16,
                                       name=f"w{j}{nm}_{g}") for g in range(4)]

        walloc(0, "q")
        walloc(0, "k")
        walloc(1, "q")
        walloc(1, "k")

        def wslice(j, nm, t):
            return wst[(j, nm)][t // 4][:, (t % 4) * HD:(t % 4 + 1) * HD]

        # small tensors ride early prio slots: the head-0 PSUM evacuations
        # need the biases, rope(0) needs cos/ss, well before the bulk
        # stream finishes
        prio = []   # (out_tile, dram_ap) in PE-need order
        for t in range(T):
            if t < 4:
                prio.append((wst[(0, "q")][t], wq_d[0][:, t * 512:(t + 1) * 512]))
                prio.append((wst[(0, "k")][t], wk_d[0][:, t * 512:(t + 1) * 512]))
            for g in range(4):
                prio.append((hT_sb[t][g], hT_d[t][:, g * 512:(g + 1) * 512]))
            if t == 0:
                prio.append((bq_sb, bq_d))
                prio.append((bk_sb, bk_d))
                prio.append((bv_sb, bv_d))
            if t == 5:      # rope tables, needed when the t-loop drains
                prio.append((cos_sb, cos_d))
                prio.append((ss_sb, ss_d))
            if t == 7:      # head-1 q weights land before the S0 hole-filler
                for g in range(4):
                    prio.append((wst[(1, "q")][g],
                                 wq_d[1][:, g * 512:(g + 1) * 512]))
            if t == 9:
                prio.append((md_sb, md_d))
            if t == 11:
                for g in range(4):
                    prio.append((wst[(1, "k")][g],
                                 wk_d[1][:, g * 512:(g + 1) * 512]))
        dma_eng = [nc.gpsimd, nc.scalar, nc.sync]
        for i, (out_t, in_ap) in enumerate(prio):
            dma_eng[i % 3].dma_start(out=out_t, in_=in_ap)

        ropep = ctx.enter_context(tc.tile_pool(name="ropep", bufs=1))
        wvp = ctx.enter_context(tc.tile_pool(name="wvp", bufs=1))
        wv_sb = wvp.tile([128, T * NHC_ * HD], f16, name="wv_sb")
        for g in range(4):
            sl = slice(g * T * NHC_ * HD // 4, (g + 1) * T * NHC_ * HD // 4)
            dma_eng[g % 3].dma_start(out=wv_sb[:, sl], in_=wv_d[:, sl])

        def load_w(j):
            for nm, w_d0 in (("q", wq_d), ("k", wk_d)):
                walloc(j, nm)
                for g in range(4):
                    nc.sync.dma_start(out=wst[(j, nm)][g],
                                      in_=w_d0[j][:, g * 512:(g + 1) * 512])

        for g in range(4):
            sl = slice(g * NHC_ * H_ // 4, (g + 1) * NHC_ * H_ // 4)
            nc.scalar.dma_start(out=wo_sb[:, sl], in_=wo_d[:, sl])

        # attnp/psm are created after the wv pool closes (SBUF/PSUM reuse);
        # attn_chunk binds them late, first use is after v-projection.

        def rope(j):
            """rope(qs_st/ks_st) -> qr/kr ring j%2 (vector + gpsimd shifts)."""
            r = j % 2
            for src, dst in ((qs_st, qr[r]), (ks_st, kr[r])):
                sh = ropep.tile([128, S_], f16, tag="sh", bufs=1, name="sh")
                acc = ropep.tile([128, S_], f16, tag="racc", bufs=1, name="racc")
                nc.gpsimd.dma_start(out=sh[0:64], in_=src[64:128])
                nc.gpsimd.dma_start(out=sh[64:128], in_=src[0:64])
                nc.vector.tensor_mul(acc, src, cos_sb)
                nc.vector.tensor_mul(dst, sh, ss_sb)
                nc.vector.tensor_add(dst, acc, dst)

        def proj_ops(j, parts=("q", "k"), pool=None):
            """PE-filler closures for head-j q/k projection (+ vector evac)."""
            for nm in parts:
                dstt = qs_st if nm == "q" else ks_st
                b_sb = bq_sb if nm == "q" else bk_sb
                for c in range(CC):
                    pp = pool.tile([128, 512], f32, tag="pp", bufs=2, name="pp")
                    for t in range(T):
                        yield lambda pp=pp, j=j, nm=nm, c=c, t=t: nc.tensor.matmul(
                            pp, lhsT=wslice(j, nm, t), rhs=hT_sb[t][c],
                            start=(t == 0), stop=(t == T - 1))
                    if zero_bias and c % 2:
                        yield lambda pp=pp, dstt=dstt, c=c: \
                            nc.scalar.activation(
                                dstt[:, c * 512:(c + 1) * 512], pp, AF.Copy)
                    elif zero_bias:
                        yield lambda pp=pp, dstt=dstt, c=c: \
                            nc.vector.tensor_copy(
                                dstt[:, c * 512:(c + 1) * 512], pp)
                    else:
                        yield lambda pp=pp, dstt=dstt, c=c, b_sb=b_sb, j=j: \
                            nc.vector.tensor_scalar_add(
                                dstt[:, c * 512:(c + 1) * 512], pp,
                                b_sb[:, j:j + 1])

        # ---------------- S0: head-0 q/k projection, t-outer ----------------
        with tc.tile_pool(name="ps0", bufs=1, space="PSUM") as ps0:
            ppq = [ps0.tile([128, 512], f32, tag="pj0", bufs=8, name=f"ppq{c}")
                   for c in range(CC)]
            ppk = [ps0.tile([128, 512], f32, tag="pj0", bufs=8, name=f"ppk{c}")
                   for c in range(CC)]
            for t in range(T):
                wq0 = wst[(0, "q")][t // 4][:, (t % 4) * HD:(t % 4 + 1) * HD]
                wk0 = wst[(0, "k")][t // 4][:, (t % 4) * HD:(t % 4 + 1) * HD]
                for c in range(CC):
                    nc.tensor.matmul(
                        ppq[c], lhsT=wq0, rhs=hT_sb[t][c],
                        start=(t == 0), stop=(t == T - 1))
                    nc.tensor.matmul(
                        ppk[c], lhsT=wk0, rhs=hT_sb[t][c],
                        start=(t == 0), stop=(t == T - 1))
            for c in range(CC):
                if zero_bias:
                    # plain copies: split across vector+scalar so the PSUM
                    # banks drain twice as fast for the next pool's matmuls
                    nc.vector.tensor_copy(
                        qs_st[:, c * 512:(c + 1) * 512], ppq[c])
                    nc.scalar.activation(
                        ks_st[:, c * 512:(c + 1) * 512], ppk[c], AF.Copy)
                else:
                    nc.vector.tensor_scalar_add(
                        qs_st[:, c * 512:(c + 1) * 512], ppq[c], bq_sb[:, 0:1])
                    nc.vector.tensor_scalar_add(
                        ks_st[:, c * 512:(c + 1) * 512], ppk[c], bk_sb[:, 0:1])

        # rope(0) runs as soon as head-0 evacs land, freeing qs_st/ks_st
        # for the later heads' projections
        rope(0)

        psm = ctx.enter_context(tc.tile_pool(name="psm", bufs=1, space="PSUM"))

        def vproj_ops(sts):
            """v-projection ops for the given s-tiles (16 mm + bias + copy)."""
            for st in sts:
                vp = psm.tile([128, NHC_ * HD], f32, tag="pp", bufs=2,
                              name="vp")
                for t in range(T):
                    yield lambda vp=vp, st=st, t=t: nc.tensor.matmul(
                        vp,
                        lhsT=hT_sb[t][st // 4][:, (st % 4) * 128:
                                               (st % 4) * 128 + 128],
                        rhs=wv_sb[:, t * NHC_ * HD:(t + 1) * NHC_ * HD],
                        start=(t == 0),
                        stop=(zero_bias and t == T - 1))
                if not zero_bias:
                    yield lambda vp=vp: nc.tensor.matmul(
                        vp, lhsT=ones1, rhs=bv_sb, start=False, stop=True)
                yield lambda vp=vp, st=st: nc.vector.tensor_copy(
                    vs[:, st * NHC_ * HD:(st + 1) * NHC_ * HD], vp)

        # ---------------- S0 tail: head-1 projection + rope + v-proj front --
        # the input stream (hT+wv) is still landing; head-1's projection and
        # the front half of the v-projection keep the PE busy through it
        for f in proj_ops(1, ("q", "k"), pool=psm):
            f()
        rope(1)
        for f in vproj_ops(range(0, 4)):
            f()

        attnp = ctx.enter_context(tc.tile_pool(name="attnp", bufs=1))

        def attn_chunk(jj, c, pull, diag_first=True):
            """Head jj's chunk-c scores/exp/au/normalize; pull() issues
            PE-filler ops between score pairs. The au accumulation is
            interleaved per pair so each exp tile dies immediately
            (expT is only [128, 1024]); the softmax denominator is a
            running vector sum into expsum."""
            rr = jj % 2
            inc = 4 * (c + 1)           # causal: k-tiles 0..4c+3
            # diag_first: masked diagonal pairs first so the chunk tail has
            # no mask dependency; diag-last when the vector queue is busy
            # early in the chunk (rope overlap)
            if diag_first:
                order = [inc - 4, inc - 2] + list(range(0, inc - 4, 2))
            else:
                order = list(range(0, inc, 2))
            au = psm.tile([128, 512], f32, tag="au", bufs=2, name="au")
            expsum = attnp.tile([128, 512], f16, tag="expsum", bufs=2,
                                name="expsum")
            prev = None
            first_pair = order[0]

            def au_and_sum(q0, qexpT):
                for kk in (0, 1):
                    kt = q0 + kk
                    nc.tensor.matmul(
                        au,
                        lhsT=vs[:, (kt * NHC_ + jj) * HD:
                                (kt * NHC_ + jj + 1) * HD],
                        rhs=qexpT[:, kk * 512:(kk + 1) * 512],
                        start=(q0 == first_pair and kk == 0),
                        stop=(q0 == order[-1] and kk == 1))
                if q0 == first_pair:
                    nc.vector.tensor_add(
                        expsum, qexpT[:, 0:512], qexpT[:, 512:1024])
                else:
                    nc.vector.tensor_add(expsum, expsum, qexpT[:, 0:512])
                    nc.vector.tensor_add(expsum, expsum, qexpT[:, 512:1024])

            for p0 in order:
                scp = psm.tile([128, 1024], f32, tag="scp", bufs=2, name="scp")
                for kk in (0, 1):
                    kt = p0 + kk
                    nc.tensor.matmul(
                        scp[:, kk * 512:(kk + 1) * 512],
                        lhsT=kr[rr][:, kt * 128:(kt + 1) * 128],
                        rhs=qr[rr][:, c * 512:(c + 1) * 512],
                        start=True, stop=True)
                expT = attnp.tile([128, 1024], f16, tag="expT", bufs=3,
                                  name="expT")
                # fused scale+exp over the pair
                nc.scalar.activation(expT, scp, AF.Exp, scale=SCALE)
                if p0 >= inc - 4:
                    # multiplicative causal mask (diag tiles p0-(inc-4), +1)
                    i0 = (p0 - (inc - 4)) * 512
                    nc.vector.tensor_mul(expT, expT, md_sb[:, i0:i0 + 1024])
                # au accumulation for the previous pair (its exp is ready)
                if prev is not None:
                    au_and_sum(*prev)
                prev = (p0, expT)
                pull()
            au_and_sum(*prev)
            # denominator broadcast into a scp-tag buffer (au keeps 2-deep
            # rotation so the divide latency is off the PE critical path)
            rlb = psm.tile([128, 1024], f32, tag="scp", bufs=2, name="rlb")
            nc.tensor.matmul(rlb[:, 0:512], lhsT=ones_sb, rhs=expsum,
                             start=True, stop=True)
            # 1/l at ~18 bits via one custom-DVE op (plenty for fp16 output)
            rl = attnp.tile([128, 512], f32, tag="rl", bufs=2, name="rl")
            nc.vector.reciprocal_approx_fast(out=rl, in_=rlb[:, 0:512])
            nc.vector.tensor_mul(attnT[c][:, jj * 512:(jj + 1) * 512], au, rl)

        # global filler queue: head-1 k, head-2 q/k, head-3 q/k projections.
        # Stage j's rope(j) requires head-j's projection fully consumed, so
        # each stage drains through its threshold before rope at chunk 2.
        from itertools import chain as _chain

        # Fill windows balance: v-proj tail (144 ops) feeds stage 1, head-2
        # projection feeds stage 2, head-3 feeds stage 3, out-proj feeds F.
        # A stage may not pull the NEXT head's ops until its rope has run
        # (single qs_st/ks_st staging buffer).
        VT = (KT - 4) * (17 if zero_bias else 18)   # v-proj tail ops
        fill_q = _chain(vproj_ops(range(4, KT)),
                        proj_ops(2, pool=psm), proj_ops(3, pool=psm))
        consumed = [0]
        limit = [0]

        def pull():
            for _ in range(12):
                if consumed[0] >= limit[0]:
                    return
                f = next(fill_q, None)
                if f is None:
                    return
                f()
                consumed[0] += 1

        def drain_until(n):
            while consumed[0] < n:
                f = next(fill_q, None)
                if f is None:
                    break
                f()
                consumed[0] += 1

        # (head, pre-rope limit, chunk-2 drain, rope head, post limit,
        #  per-chunk pre-drains) -- stage 1's pre-drains guarantee the vs
        # tiles each chunk's au needs have been produced; caps keep the next
        # head's evacuations behind the rope that frees qs_st/ks_st.
        plan = ((0, VT + 136, VT, None, VT + 136, (0, 72, 144, 216)),
                (1, VT + 136, VT + 136, 2, VT + 204, None),
                (2, VT + 272, VT + 272, 3, VT + 272, None))
        for jj, lim, drain_at, rope_j, post, pre in plan:
            if jj + 2 < NHC_:
                load_w(jj + 2)
            limit[0] = lim
            for c in range(CC):
                if pre is not None and pre[c]:
                    drain_until(pre[c])
                attn_chunk(jj, c, pull, diag_first=(c != CC - 1))
                if c == CC - 2:
                    drain_until(drain_at)
                    if rope_j is not None:
                        rope(rope_j)
                    limit[0] = post

        # ---------------- final: head-3 attention + output projection ----------------
        def outproj_chunk(c):
            """Output-projection ops for chunk c (needs attnT[c] complete)."""
            for st in range(4):
                for half in range(2):
                    osb = attnp.tile([128, 1024], f16, tag="osb", bufs=2,
                                     name="osb")
                    for hh in range(2):
                        hc = half * 2 + hh
                        op = psm.tile([128, 512], f32, tag="pp", bufs=2,
                                      name="op")
                        for jj in range(NHC_):
                            yield lambda op=op, c=c, st=st, hc=hc, jj=jj: \
                                nc.tensor.matmul(
                                    op,
                                    lhsT=attnT[c][:, jj * 512 + st * 128:
                                                  jj * 512 + st * 128 + 128],
                                    rhs=wo_sb[:, jj * H_ + hc * 512:
                                              jj * H_ + hc * 512 + 512],
                                    start=(jj == 0), stop=(jj == NHC_ - 1))
                        # alternate PSUM evacuation between vector and scalar
                        if hc % 2 == 0:
                            yield lambda op=op, osb=osb, hh=hh: \
                                nc.vector.tensor_copy(
                                    osb[:, hh * 512:(hh + 1) * 512], op)
                        else:
                            yield lambda op=op, osb=osb, hh=hh: \
                                nc.scalar.activation(
                                    osb[:, hh * 512:(hh + 1) * 512], op,
                                    AF.Copy)

                    def store(osb=osb, c=c, st=st, half=half):
                        row = c * 512 + st * 128
                        nc.sync.dma_start(
                            out=o_d[row: row + 128,
                                    half * 1024:(half + 1) * 1024],
                            in_=osb)
                    yield store

        dq = deque()

        def pull_dq():
            for _ in range(12):
                if not dq:
                    return
                dq.popleft()()

        for c in range(CC):
            attn_chunk(NHC_ - 1, c, pull_dq)
            dq.extend(outproj_chunk(c))
        while dq:
            dq.popleft()()

    nc.compile()
    return nc


def prep_core_inputs(hidden_b, mask_b, Wq, bq, Wk, bk, Wv, bv, Wo, n0, S_, H_,
                     NHC_, cosT, ssT, mdiag):
    """Host-side prep of one core's input map. hidden_b [S,H] f32."""
    T = H_ // 128
    f16 = np.float16

    hT = np.ascontiguousarray(hidden_b.T).reshape(T, 128, S_).astype(f16)

    def w_slices(W):
        # [H, NH, HD] -> [NHC, 128, T*HD]
        out = np.empty((NHC_, 128, T * HD), f16)
        for j in range(NHC_):
            w = W[:, n0 + j, :].reshape(T, 128, HD)         # [t, p, d]
            out[j] = w.transpose(1, 0, 2).reshape(128, T * HD)
        return out

    wq = w_slices(Wq)
    wk = w_slices(Wk)
    # pre-transposed into SBUF layout for contiguous DMA
    wv = np.ascontiguousarray(
        Wv[:, n0:n0 + NHC_, :].reshape(T, 128, NHC_ * HD).transpose(1, 0, 2)
    ).reshape(128, T * NHC_ * HD).astype(f16)
    wo = np.ascontiguousarray(
        Wo[n0:n0 + NHC_].transpose(1, 0, 2)
    ).reshape(128, NHC_ * H_).astype(f16)

    bqT = np.ascontiguousarray(bq[n0:n0 + NHC_].T).astype(np.float32)
    bkT = np.ascontiguousarray(bk[n0:n0 + NHC_].T).astype(np.float32)
    bv4 = bv[n0:n0 + NHC_].reshape(1, NHC_ * HD).astype(f16)

    return {
        "hT": hT, "wq": wq, "wk": wk, "wv": wv, "wo": wo,
        "cosT": cosT, "ssT": ssT, "bqT": bqT, "bkT": bkT, "bv4": bv4,
        "mdiag": mdiag,
    }


def _check_causal_and_diag(mask):
    """Verify the mask is the standard causal mask (shared by all batches)
    and build the shared [128, 4*512] diagonal block in [k, q] layout."""
    S_ = mask.shape[-1]
    m01 = (mask[0, 0] <= 0.5)                      # True where attention allowed
    # diagonal block from chunk 0: rows k 0..511, cols q 0..511, [k, q] layout
    blk = m01[:512, :512].T.copy()                 # [k, q] -> wait: m01 is [q, k]
    # m01[q, k]: allowed = k <= q. Transposed to [k, q]:
    mT = m01.T                                     # [k, q]
    blk = mT[:512, :512]                           # [k, q] diagonal block
    mdiag = np.ascontiguousarray(
        blk.reshape(4, 128, 512).transpose(1, 0, 2).reshape(128, 4 * 512)
    ).astype(np.float16)
    # verify causal structure cheaply: the full mask must equal k <= q
    q_idx = np.arange(S_)
    expect_rows = [0, 1, 511, 512, 1000, 2047]
    for r in expect_rows:
        if not np.array_equal(m01[r], q_idx <= r):
            raise ValueError("mask is not the standard causal mask")
    for b in range(mask.shape[0]):
        if not np.array_equal((mask[b, 0] <= 0.5), m01):
            raise ValueError("mask differs across batches")
    return mdiag


def kernel(hidden_states, mask, Wq, bq, Wk, bk, Wv, bv, Wo, bo):
    global LAST_RESULTS
    from concourse.bass_utils import run_bass_kernel_spmd

    hidden_states = np.asarray(hidden_states, dtype=np.float32)
    mask = np.asarray(mask, dtype=np.float32)
    Wq, bq = np.asarray(Wq, np.float32), np.asarray(bq, np.float32)
    Wk, bk = np.asarray(Wk, np.float32), np.asarray(bk, np.float32)
    Wv, bv = np.asarray(Wv, np.float32), np.asarray(bv, np.float32)
    Wo, bo = np.asarray(Wo, np.float32), np.asarray(bo, np.float32)

    cosT, ssT = _rope_tables(S)
    mdiag = _check_causal_and_diag(mask)

    in_maps = []
    for core in range(N_CORES):
        b = core // HGRID
        n0 = (core % HGRID) * NHC
        in_maps.append(prep_core_inputs(
            hidden_states[b], mask[b, 0], Wq, bq, Wk, bk, Wv, bv, Wo,
            n0, S, H, NHC, cosT, ssT, mdiag))

    zb = not (bq.any() or bk.any() or bv.any())
    key = (S, H, NHC, zb)
    if key not in _CACHE:
        _CACHE[key] = build_program(S, H, NHC, zero_bias=zb)
    nc = _CACHE[key]

    res = run_bass_kernel_spmd(nc, in_maps, core_ids=list(range(N_CORES)))
    LAST_RESULTS = res

    out = np.zeros((B, S, H), np.float32)
    for core in range(N_CORES):
        out[core // HGRID] += res.results[core]["o"].astype(np.float32)
    out += bo[None, None, :]
    return out


# revision 43
# speedup vs baseline: 1.1880x; 1.0147x over previous
"""Trainium2 Bass kernel for multi-head self-attention with RoPE.

Sharding: 8 cores = 2 (batch) x 4 (head groups of 4 heads).
Each core computes its batch's attention for its 4 heads plus the
(row-sharded) output projection partial sum; the host adds the 4 head-group
partials per batch and the output bias.

Schedule (single Tile program, engines self-synchronize):
  S0:      head-0 q/k projection t-outer across 8 PSUM banks so the PE
           consumes hidden-state tiles as the DMAs land; then v projection.
  stage j: head j's q/k projection matmuls are interleaved between head
           j-1's score matmuls as PE filler while the scalar engine
           computes exp() of the score tiles; per (chunk, head) the softmax
           denominator is a vector-engine tree-sum of the exp tiles plus a
           single ones-matmul broadcast, inverted with vector reciprocal
           (no Ln -> single ACT table, no table thrash).
  final:   head-3 attention chunk-by-chunk with the previous chunk's
           output-projection matmuls as PE filler.

The causal mask multiply uses one shared [128, 4*512] diagonal block
(identical for every query chunk) applied as a single tensor_mul per
(chunk, head). Output is stored fp16; host adds partials in fp32.
"""

import sys
import types
from collections import deque

import numpy as np

sys.path.insert(0, "/opt/trn_rl_repo")

# The axon boot registers its NTFF-profiling hook via antenv.axon_hooks; some
# images lack that module, which silently disables tracing. Provide it.
if "antenv.axon_hooks" not in sys.modules:
    try:
        import antenv.axon_hooks  # noqa: F401
    except ImportError:
        try:
            import antenv

            _m = types.ModuleType("antenv.axon_hooks")
            _m._hook = None
            _m.set_axon_ntff_profile_hook = lambda h: setattr(_m, "_hook", h)
            _m.get_axon_ntff_profile_hook = lambda: _m._hook
            sys.modules["antenv.axon_hooks"] = _m
            antenv.axon_hooks = _m
        except ImportError:
            pass

B, S, H, NH, HD = 2, 2048, 2048, 16, 128
ROPE_THETA = 10000.0
N_CORES = 8
HGRID = 4            # head-group shards
NHC = NH // HGRID    # heads per core

LAST_RESULTS = None  # test harness introspection
_CACHE = {}


def _rope_tables(S_, dtype=np.float16):
    # transposed rope tables [HD, S]; ss has rotate-half sign folded in:
    # rope(x)[d, s] = x[d, s]*cosT[d, s] + x[(d+64)%128, s]*ss[d, s]
    inv = 1.0 / (ROPE_THETA ** (np.arange(0, HD, 2, dtype=np.float64) / HD))
    t = np.arange(S_, dtype=np.float64)
    fr = np.outer(t, inv)                          # [S, HD/2]
    emb = np.concatenate([fr, fr], axis=1)         # [S, HD]
    cosT = np.cos(emb).T.astype(np.float32)        # [HD, S]
    ss = np.sin(emb).T.astype(np.float32)
    ss[: HD // 2] *= -1.0
    return cosT.astype(dtype), ss.astype(dtype)


def build_program(S_, H_, NHC_, zero_bias=False):
    """Build + compile the per-core SPMD bass program (causal schedule)."""
    from contextlib import ExitStack

    import concourse.mybir as mybir
    import concourse.tile as tile
    from concourse import bacc

    f16 = mybir.dt.float16
    f32 = mybir.dt.float32
    AF = mybir.ActivationFunctionType

    T = H_ // 128       # hidden contraction tiles
    KT = S_ // 128      # key/seq tiles
    CC = S_ // 512      # query chunks
    HC = H_ // 512      # output hidden chunks
    SCALE = 1.0 / float(np.sqrt(HD))

    nc = bacc.Bacc("TRN2", target_bir_lowering=False, debug=False)

    hT_d = nc.dram_tensor("hT", [T, 128, S_], f16, kind="ExternalInput").ap()
    wq_d = nc.dram_tensor("wq", [NHC_, 128, T * HD], f16, kind="ExternalInput").ap()
    wk_d = nc.dram_tensor("wk", [NHC_, 128, T * HD], f16, kind="ExternalInput").ap()
    # wv/wo are pre-transposed host-side into SBUF layout (contiguous DMA)
    wv_d = nc.dram_tensor("wv", [128, T * NHC_ * HD], f16, kind="ExternalInput").ap()
    wo_d = nc.dram_tensor("wo", [128, NHC_ * H_], f16, kind="ExternalInput").ap()
    cos_d = nc.dram_tensor("cosT", [128, S_], f16, kind="ExternalInput").ap()
    ss_d = nc.dram_tensor("ssT", [128, S_], f16, kind="ExternalInput").ap()
    bq_d = nc.dram_tensor("bqT", [128, NHC_], f32, kind="ExternalInput").ap()
    bk_d = nc.dram_tensor("bkT", [128, NHC_], f32, kind="ExternalInput").ap()
    bv_d = nc.dram_tensor("bv4", [1, NHC_ * HD], f16, kind="ExternalInput").ap()
    md_d = nc.dram_tensor("mdiag", [128, 4 * 512], f16, kind="ExternalInput").ap()
    o_d = nc.dram_tensor("o", [S_, H_], f16, kind="ExternalOutput").ap()

    with ExitStack() as ctx:
        tc = ctx.enter_context(tile.TileContext(nc))
        persist = ctx.enter_context(tc.tile_pool(name="persist", bufs=1))

        # qr/kr 2-deep rings: stage j writes ring[j % 2]; head j's attention
        # reads it during stage j+1.
        qr = [persist.tile([128, S_], f16, name=f"qr{r}") for r in range(2)]
        kr = [persist.tile([128, S_], f16, name=f"kr{r}") for r in range(2)]
        vs = persist.tile([128, KT * NHC_ * HD], f16, name="vs")
        wo_sb = persist.tile([128, NHC_ * H_], f16, name="wo_sb")
        attnT = [persist.tile([128, NHC_ * 512], f16, name=f"attnT{c}")
                 for c in range(CC)]
        cos_sb = persist.tile([128, S_], f16, name="cos_sb")
        ss_sb = persist.tile([128, S_], f16, name="ss_sb")
        md_sb = persist.tile([128, 4 * 512], f16, name="md_sb")
        ones_sb = persist.tile([128, 128], f16, name="ones_sb")
        ones1 = persist.tile([1, 128], f16, name="ones1")
        bv_sb = persist.tile([1, NHC_ * HD], f16, name="bv_sb")
        bq_sb = persist.tile([128, NHC_], f32, name="bq_sb")
        bk_sb = persist.tile([128, NHC_], f32, name="bk_sb")
        # single staging buffer for pre-rope q/k (vector-queue order makes
        # reuse across stages safe: rope(j) reads are issued before stage
        # j+1's evac writes on the same queue)
        qs_st = persist.tile([128, S_], f16, name="qs_st")
        ks_st = persist.tile([128, S_], f16, name="ks_st")

        nc.vector.memset(ones_sb, 1.0)
        nc.vector.memset(ones1, 1.0)

        # per-head q/k weight tiles, 4-deep rotation (j and j+1 in flight)
        wpool = ctx.enter_context(tc.tile_pool(name="wpool", bufs=1))

        # ---- DMA issue (3 dynamic queues: sync, gpsimd, scalar) ----
        # Startup transfers are quartered (128KB) and dealt round-robin to
        # the three queues in PE-need order, so the first matmul's inputs
        # land within ~1us of queue start and the t-loop streams.
        hT_pool = ctx.enter_context(tc.tile_pool(name="hTp", bufs=1))
        # hT_sb[t][g]: quarter tiles [128, 512] (g = column group)
        hT_sb = [[hT_pool.tile([128, 512], f16, name=f"hT{t}_{g}")
                  for g in range(4)] for t in range(T)]
        # head-0 q/k weights quartered: w0x[g] holds t-slices 4g..4g+3
        # all q/k weights stored as 4 quarter-tiles [128, 512] per (head,
        # q/k), sharing one 16-deep rotation (two heads in flight)
        wst = {}

        def walloc(j, nm):
            wst[(j, nm)] = [wpool.tile([128, 512], f16, tag="w", bufs=16,
                                       name=f"w{j}{nm}_{g}") for g in range(4)]

        walloc(0, "q")
        walloc(0, "k")
        walloc(1, "q")
        walloc(1, "k")

        def wslice(j, nm, t):
            return wst[(j, nm)][t // 4][:, (t % 4) * HD:(t % 4 + 1) * HD]

        # small tensors ride early prio slots: the head-0 PSUM evacuations
        # need the biases, rope(0) needs cos/ss, well before the bulk
        # stream finishes
        prio = []   # (out_tile, dram_ap) in PE-need order
        for t in range(T):
            if t < 4:
                prio.append((wst[(0, "q")][t], wq_d[0][:, t * 512:(t + 1) * 512]))
                prio.append((wst[(0, "k")][t], wk_d[0][:, t * 512:(t + 1) * 512]))
            for g in range(4):
                prio.append((hT_sb[t][g], hT_d[t][:, g * 512:(g + 1) * 512]))
            if t == 0:
                prio.append((bq_sb, bq_d))
                prio.append((bk_sb, bk_d))
                prio.append((bv_sb, bv_d))
            if t == 5:      # rope tables, needed when the t-loop drains
                prio.append((cos_sb, cos_d))
                prio.append((ss_sb, ss_d))
            if t == 7:      # head-1 q weights land before the S0 hole-filler
                for g in range(4):
                    prio.append((wst[(1, "q")][g],
                                 wq_d[1][:, g * 512:(g + 1) * 512]))
            if t == 9:
                prio.append((md_sb, md_d))
            if t == 11:
                for g in range(4):
                    prio.append((wst[(1, "k")][g],
                                 wk_d[1][:, g * 512:(g + 1) * 512]))
        dma_eng = [nc.gpsimd, nc.scalar, nc.sync]
        for i, (out_t, in_ap) in enumerate(prio):
            dma_eng[i % 3].dma_start(out=out_t, in_=in_ap)

        ropep = ctx.enter_context(tc.tile_pool(name="ropep", bufs=1))
        wvp = ctx.enter_context(tc.tile_pool(name="wvp", bufs=1))
        wv_sb = wvp.tile([128, T * NHC_ * HD], f16, name="wv_sb")
        for g in range(4):
            sl = slice(g * T * NHC_ * HD // 4, (g + 1) * T * NHC_ * HD // 4)
            dma_eng[g % 3].dma_start(out=wv_sb[:, sl], in_=wv_d[:, sl])

        def load_w(j):
            for nm, w_d0 in (("q", wq_d), ("k", wk_d)):
                walloc(j, nm)
                for g in range(4):
                    nc.sync.dma_start(out=wst[(j, nm)][g],
                                      in_=w_d0[j][:, g * 512:(g + 1) * 512])

        for g in range(4):
            sl = slice(g * NHC_ * H_ // 4, (g + 1) * NHC_ * H_ // 4)
            nc.scalar.dma_start(out=wo_sb[:, sl], in_=wo_d[:, sl])

        # attnp/psm are created after the wv pool closes (SBUF/PSUM reuse);
        # attn_chunk binds them late, first use is after v-projection.

        def rope(j):
            """rope(qs_st/ks_st) -> qr/kr ring j%2 (vector + gpsimd shifts)."""
            r = j % 2
            for src, dst in ((qs_st, qr[r]), (ks_st, kr[r])):
                sh = ropep.tile([128, S_], f16, tag="sh", bufs=1, name="sh")
                acc = ropep.tile([128, S_], f16, tag="racc", bufs=1, name="racc")
                nc.gpsimd.dma_start(out=sh[0:64], in_=src[64:128])
                nc.gpsimd.dma_start(out=sh[64:128], in_=src[0:64])
                nc.vector.tensor_mul(acc, src, cos_sb)
                nc.vector.tensor_mul(dst, sh, ss_sb)
                nc.vector.tensor_add(dst, acc, dst)

        def proj_ops(j, parts=("q", "k"), pool=None):
            """PE-filler closures for head-j q/k projection (+ vector evac)."""
            for nm in parts:
                dstt = qs_st if nm == "q" else ks_st
                b_sb = bq_sb if nm == "q" else bk_sb
                for c in range(CC):
                    pp = pool.tile([128, 512], f32, tag="pp", bufs=2, name="pp")
                    for t in range(T):
                        yield lambda pp=pp, j=j, nm=nm, c=c, t=t: nc.tensor.matmul(
                            pp, lhsT=wslice(j, nm, t), rhs=hT_sb[t][c],
                            start=(t == 0), stop=(t == T - 1))
                    if zero_bias and c % 2:
                        yield lambda pp=pp, dstt=dstt, c=c: \
                            nc.scalar.activation(
                                dstt[:, c * 512:(c + 1) * 512], pp, AF.Copy)
                    elif zero_bias:
                        yield lambda pp=pp, dstt=dstt, c=c: \
                            nc.vector.tensor_copy(
                                dstt[:, c * 512:(c + 1) * 512], pp)
                    else:
                        yield lambda pp=pp, dstt=dstt, c=c, b_sb=b_sb, j=j: \
                            nc.vector.tensor_scalar_add(
                                dstt[:, c * 512:(c + 1) * 512], pp,
                                b_sb[:, j:j + 1])

        # ---------------- S0: head-0 q/k projection, t-outer ----------------
        with tc.tile_pool(name="ps0", bufs=1, space="PSUM") as ps0:
            ppq = [ps0.tile([128, 512], f32, tag="pj0", bufs=8, name=f"ppq{c}")
                   for c in range(CC)]
            ppk = [ps0.tile([128, 512], f32, tag="pj0", bufs=8, name=f"ppk{c}")
                   for c in range(CC)]
            for t in range(T):
                wq0 = wst[(0, "q")][t // 4][:, (t % 4) * HD:(t % 4 + 1) * HD]
                wk0 = wst[(0, "k")][t // 4][:, (t % 4) * HD:(t % 4 + 1) * HD]
                for c in range(CC):
                    nc.tensor.matmul(
                        ppq[c], lhsT=wq0, rhs=hT_sb[t][c],
                        start=(t == 0), stop=(t == T - 1))
                    nc.tensor.matmul(
                        ppk[c], lhsT=wk0, rhs=hT_sb[t][c],
                        start=(t == 0), stop=(t == T - 1))
            for c in range(CC):
                if zero_bias:
                    # plain copies: split across vector+scalar so the PSUM
                    # banks drain twice as fast for the next pool's matmuls
                    nc.vector.tensor_copy(
                        qs_st[:, c * 512:(c + 1) * 512], ppq[c])
                    nc.scalar.activation(
                        ks_st[:, c * 512:(c + 1) * 512], ppk[c], AF.Copy)
                else:
                    nc.vector.tensor_scalar_add(
                        qs_st[:, c * 512:(c + 1) * 512], ppq[c], bq_sb[:, 0:1])
                    nc.vector.tensor_scalar_add(
                        ks_st[:, c * 512:(c + 1) * 512], ppk[c], bk_sb[:, 0:1])

        # rope(0) runs as soon as head-0 evacs land, freeing qs_st/ks_st
        # for the later heads' projections
        rope(0)

        psm = ctx.enter_context(tc.tile_pool(name="psm", bufs=1, space="PSUM"))

        def vproj_ops(sts):
            """v-projection ops for the given s-tiles (16 mm + bias + copy)."""
            for st in sts:
                vp = psm.tile([128, NHC_ * HD], f32, tag="pp", bufs=2,
                              name="vp")
                for t in range(T):
                    yield lambda vp=vp, st=st, t=t: nc.tensor.matmul(
                        vp,
                        lhsT=hT_sb[t][st // 4][:, (st % 4) * 128:
                                               (st % 4) * 128 + 128],
                        rhs=wv_sb[:, t * NHC_ * HD:(t + 1) * NHC_ * HD],
                        start=(t == 0),
                        stop=(zero_bias and t == T - 1))
                if not zero_bias:
                    yield lambda vp=vp: nc.tensor.matmul(
                        vp, lhsT=ones1, rhs=bv_sb, start=False, stop=True)
                yield lambda vp=vp, st=st: nc.vector.tensor_copy(
                    vs[:, st * NHC_ * HD:(st + 1) * NHC_ * HD], vp)

        # ---------------- S0 tail: head-1 projection + rope + v-proj front --
        # the input stream (hT+wv) is still landing; head-1's projection and
        # the front half of the v-projection keep the PE busy through it
        for f in proj_ops(1, ("q", "k"), pool=psm):
            f()
        rope(1)
        for f in vproj_ops(range(0, 4)):
            f()

        attnp = ctx.enter_context(tc.tile_pool(name="attnp", bufs=1))

        def attn_chunk(jj, c, pull, diag_first=True):
            """Head jj's chunk-c scores/exp/au/normalize; pull() issues
            PE-filler ops between score pairs. The au accumulation is
            interleaved per pair so each exp tile dies immediately
            (expT is only [128, 1024]); the softmax denominator is a
            running vector sum into expsum."""
            rr = jj % 2
            inc = 4 * (c + 1)           # causal: k-tiles 0..4c+3
            # diag_first: masked diagonal pairs first so the chunk tail has
            # no mask dependency; diag-last when the vector queue is busy
            # early in the chunk (rope overlap)
            if diag_first:
                order = [inc - 4, inc - 2] + list(range(0, inc - 4, 2))
            else:
                order = list(range(0, inc, 2))
            au = psm.tile([128, 512], f32, tag="au", bufs=2, name="au")
            expsum = attnp.tile([128, 512], f16, tag="expsum", bufs=2,
                                name="expsum")
            prev = None
            first_pair = order[0]

            def au_and_sum(q0, qexpT):
                for kk in (0, 1):
                    kt = q0 + kk
                    nc.tensor.matmul(
                        au,
                        lhsT=vs[:, (kt * NHC_ + jj) * HD:
                                (kt * NHC_ + jj + 1) * HD],
                        rhs=qexpT[:, kk * 512:(kk + 1) * 512],
                        start=(q0 == first_pair and kk == 0),
                        stop=(q0 == order[-1] and kk == 1))
                if q0 == first_pair:
                    nc.vector.tensor_add(
                        expsum, qexpT[:, 0:512], qexpT[:, 512:1024])
                else:
                    nc.vector.tensor_add(expsum, expsum, qexpT[:, 0:512])
                    nc.vector.tensor_add(expsum, expsum, qexpT[:, 512:1024])

            for p0 in order:
                scp = psm.tile([128, 1024], f32, tag="scp", bufs=2, name="scp")
                for kk in (0, 1):
                    kt = p0 + kk
                    nc.tensor.matmul(
                        scp[:, kk * 512:(kk + 1) * 512],
                        lhsT=kr[rr][:, kt * 128:(kt + 1) * 128],
                        rhs=qr[rr][:, c * 512:(c + 1) * 512],
                        start=True, stop=True)
                expT = attnp.tile([128, 1024], f16, tag="expT", bufs=3,
                                  name="expT")
                # fused scale+exp over the pair
                nc.scalar.activation(expT, scp, AF.Exp, scale=SCALE)
                if p0 >= inc - 4:
                    # multiplicative causal mask (diag tiles p0-(inc-4), +1)
                    i0 = (p0 - (inc - 4)) * 512
                    nc.vector.tensor_mul(expT, expT, md_sb[:, i0:i0 + 1024])
                # au accumulation for the previous pair (its exp is ready)
                if prev is not None:
                    au_and_sum(*prev)
                prev = (p0, expT)
                pull()
            au_and_sum(*prev)
            # denominator broadcast into a scp-tag buffer (au keeps 2-deep
            # rotation so the divide latency is off the PE critical path)
            rlb = psm.tile([128, 1024], f32, tag="scp", bufs=2, name="rlb")
            nc.tensor.matmul(rlb[:, 0:512], lhsT=ones_sb, rhs=expsum,
                             start=True, stop=True)
            # 1/l at ~18 bits via one custom-DVE op (plenty for fp16 output)
            rl = attnp.tile([128, 512], f32, tag="rl", bufs=2, name="rl")
            nc.vector.reciprocal_approx_fast(out=rl, in_=rlb[:, 0:512])
            nc.vector.tensor_mul(attnT[c][:, jj * 512:(jj + 1) * 512], au, rl)

        # global filler queue: head-1 k, head-2 q/k, head-3 q/k projections.
        # Stage j's rope(j) requires head-j's projection fully consumed, so
        # each stage drains through its threshold before rope at chunk 2.
        from itertools import chain as _chain

        # Fill windows balance: v-proj tail (144 ops) feeds stage 1, head-2
        # projection feeds stage 2, head-3 feeds stage 3, out-proj feeds F.
        # A stage may not pull the NEXT head's ops until its rope has run
        # (single qs_st/ks_st staging buffer).
        VT = (KT - 4) * (17 if zero_bias else 18)   # v-proj tail ops
        fill_q = _chain(vproj_ops(range(4, KT)),
                        proj_ops(2, pool=psm), proj_ops(3, pool=psm))
        consumed = [0]
        limit = [0]

        def pull():
            for _ in range(12):
                if consumed[0] >= limit[0]:
                    return
                f = next(fill_q, None)
                if f is None:
                    return
                f()
                consumed[0] += 1

        def drain_until(n):
            while consumed[0] < n:
                f = next(fill_q, None)
                if f is None:
                    break
                f()
                consumed[0] += 1

        # (head, pre-rope limit, chunk-2 drain, rope head, post limit,
        #  per-chunk pre-drains) -- stage 1's pre-drains guarantee the vs
        # tiles each chunk's au needs have been produced; caps keep the next
        # head's evacuations behind the rope that frees qs_st/ks_st.
        G = VT // 12                 # ops per v-proj tail group
        plan = ((0, VT + 136, VT, None, VT + 136,
                 (0, 4 * G, 8 * G, VT)),
                (1, VT + 136, VT + 136, 2, VT + 187, None),
                (2, VT + 272, VT + 272, 3, VT + 272, None))
        for jj, lim, drain_at, rope_j, post, pre in plan:
            if jj + 2 < NHC_:
                load_w(jj + 2)
            limit[0] = lim
            for c in range(CC):
                if pre is not None and pre[c]:
                    drain_until(pre[c])
                attn_chunk(jj, c, pull, diag_first=(c != CC - 1))
                if c == CC - 2:
                    drain_until(drain_at)
                    if rope_j is not None:
                        rope(rope_j)
                    limit[0] = post

        # ---------------- final: head-3 attention + output projection ----------------
        def outproj_chunk(c):
            """Output-projection ops for chunk c (needs attnT[c] complete)."""
            for st in range(4):
                for half in range(2):
                    osb = attnp.tile([128, 1024], f16, tag="osb", bufs=2,
                                     name="osb")
                    for hh in range(2):
                        hc = half * 2 + hh
                        op = psm.tile([128, 512], f32, tag="pp", bufs=2,
                                      name="op")
                        for jj in range(NHC_):
                            yield lambda op=op, c=c, st=st, hc=hc, jj=jj: \
                                nc.tensor.matmul(
                                    op,
                                    lhsT=attnT[c][:, jj * 512 + st * 128:
                                                  jj * 512 + st * 128 + 128],
                                    rhs=wo_sb[:, jj * H_ + hc * 512:
                                              jj * H_ + hc * 512 + 512],
                                    start=(jj == 0), stop=(jj == NHC_ - 1))
                        # alternate PSUM evacuation between vector and scalar
                        if hc % 2 == 0:
                            yield lambda op=op, osb=osb, hh=hh: \
                                nc.vector.tensor_copy(
                                    osb[:, hh * 512:(hh + 1) * 512], op)
                        else:
                            yield lambda op=op, osb=osb, hh=hh: \
                                nc.scalar.activation(
                                    osb[:, hh * 512:(hh + 1) * 512], op,
                                    AF.Copy)

                    def store(osb=osb, c=c, st=st, half=half):
                        row = c * 512 + st * 128
                        nc.sync.dma_start(
                            out=o_d[row: row + 128,
                                    half * 1024:(half + 1) * 1024],
                            in_=osb)
                    yield store

        dq = deque()

        def pull_dq():
            for _ in range(12):
                if not dq:
                    return
                dq.popleft()()

        for c in range(CC):
            attn_chunk(NHC_ - 1, c, pull_dq)
            dq.extend(outproj_chunk(c))
        while dq:
            dq.popleft()()

    nc.compile()
    return nc


def prep_core_inputs(hidden_b, mask_b, Wq, bq, Wk, bk, Wv, bv, Wo, n0, S_, H_,
                     NHC_, cosT, ssT, mdiag):
    """Host-side prep of one core's input map. hidden_b [S,H] f32."""
    T = H_ // 128
    f16 = np.float16

    hT = np.ascontiguousarray(hidden_b.T).reshape(T, 128, S_).astype(f16)

    def w_slices(W):
        # [H, NH, HD] -> [NHC, 128, T*HD]
        out = np.empty((NHC_, 128, T * HD), f16)
        for j in range(NHC_):
            w = W[:, n0 + j, :].reshape(T, 128, HD)         # [t, p, d]
            out[j] = w.transpose(1, 0, 2).reshape(128, T * HD)
        return out

    wq = w_slices(Wq)
    wk = w_slices(Wk)
    # pre-transposed into SBUF layout for contiguous DMA
    wv = np.ascontiguousarray(
        Wv[:, n0:n0 + NHC_, :].reshape(T, 128, NHC_ * HD).transpose(1, 0, 2)
    ).reshape(128, T * NHC_ * HD).astype(f16)
    wo = np.ascontiguousarray(
        Wo[n0:n0 + NHC_].transpose(1, 0, 2)
    ).reshape(128, NHC_ * H_).astype(f16)

    bqT = np.ascontiguousarray(bq[n0:n0 + NHC_].T).astype(np.float32)
    bkT = np.ascontiguousarray(bk[n0:n0 + NHC_].T).astype(np.float32)
    bv4 = bv[n0:n0 + NHC_].reshape(1, NHC_ * HD).astype(f16)

    return {
        "hT": hT, "wq": wq, "wk": wk, "wv": wv, "wo": wo,
        "cosT": cosT, "ssT": ssT, "bqT": bqT, "bkT": bkT, "bv4": bv4,
        "mdiag": mdiag,
    }


def _check_causal_and_diag(mask):
    """Verify the mask is the standard causal mask (shared by all batches)
    and build the shared [128, 4*512] diagonal block in [k, q] layout."""
    S_ = mask.shape[-1]
    m01 = (mask[0, 0] <= 0.5)                      # True where attention allowed
    # diagonal block from chunk 0: rows k 0..511, cols q 0..511, [k, q] layout
    blk = m01[:512, :512].T.copy()                 # [k, q] -> wait: m01 is [q, k]
    # m01[q, k]: allowed = k <= q. Transposed to [k, q]:
    mT = m01.T                                     # [k, q]
    blk = mT[:512, :512]                           # [k, q] diagonal block
    mdiag = np.ascontiguousarray(
        blk.reshape(4, 128, 512).transpose(1, 0, 2).reshape(128, 4 * 512)
    ).astype(np.float16)
    # verify causal structure cheaply: the full mask must equal k <= q
    q_idx = np.arange(S_)
    expect_rows = [0, 1, 511, 512, 1000, 2047]
    for r in expect_rows:
        if not np.array_equal(m01[r], q_idx <= r):
            raise ValueError("mask is not the standard causal mask")
    for b in range(mask.shape[0]):
        if not np.array_equal((mask[b, 0] <= 0.5), m01):
            raise ValueError("mask differs across batches")
    return mdiag


def kernel(hidden_states, mask, Wq, bq, Wk, bk, Wv, bv, Wo, bo):
    global LAST_RESULTS
    from concourse.bass_utils import run_bass_kernel_spmd

    hidden_states = np.asarray(hidden_states, dtype=np.float32)
    mask = np.asarray(mask, dtype=np.float32)
    Wq, bq = np.asarray(Wq, np.float32), np.asarray(bq, np.float32)
    Wk, bk = np.asarray(Wk, np.float32), np.asarray(bk, np.float32)
    Wv, bv = np.asarray(Wv, np.float32), np.asarray(bv, np.float32)
    Wo, bo = np.asarray(Wo, np.float32), np.asarray(bo, np.float32)

    cosT, ssT = _rope_tables(S)
    mdiag = _check_causal_and_diag(mask)

    in_maps = []
    for core in range(N_CORES):
        b = core // HGRID
        n0 = (core % HGRID) * NHC
        in_maps.append(prep_core_inputs(
            hidden_states[b], mask[b, 0], Wq, bq, Wk, bk, Wv, bv, Wo,
            n0, S, H, NHC, cosT, ssT, mdiag))

    zb = not (bq.any() or bk.any() or bv.any())
    key = (S, H, NHC, zb)
    if key not in _CACHE:
        _CACHE[key] = build_program(S, H, NHC, zero_bias=zb)
    nc = _CACHE[key]

    res = run_bass_kernel_spmd(nc, in_maps, core_ids=list(range(N_CORES)))
    LAST_RESULTS = res

    out = np.zeros((B, S, H), np.float32)
    for core in range(N_CORES):
        out[core // HGRID] += res.results[core]["o"].astype(np.float32)
    out += bo[None, None, :]
    return out
